# revision 1
# baseline (speedup 1.0000x reference)
"""Trainium2 Bass kernel for nn_BCA_17274358465235.

Module: out = x + conv1x1_up( softmax(fx @ fy_up^T) @ fself ) with
fx/fself = 2-layer 1x1-conv projections of x, fy = projection of
bilinearly-upsampled y.  B=4, CX=256, CY=512, CM=64, H=W=64 (N=4096
tokens), HY=WY=32.

Sharding: 8 cores = batch(4) x query-row-half(2).  Each core holds all
4096 keys (fy/fself replicated per batch) and 2048 query rows.  No
collectives.

Per-core algorithm (layouts chosen so no transposes are needed):
  fself^T[key, c]  via second projection layer emitted transposed
  sim^T[key, row] = fy_f[:, keys]^T @ fx[:, rows]   (fp32r matmuls,
      two key-chunks packed into PE row-groups 0-1 / 2-3)
  exp on ACT (no max-subtraction: |sim| < 70, fp32-safe)
  fout^T[c, row] += fself^T_chunk^T @ exp_chunk   (PSUM accumulation,
      ones-column in fself^T produces the softmax denominator Z free)
  out = x + W_up @ (fout^T * (1/Z)) + b_up   (b_up via ones-row in W_up)
The y-upsample runs after the channel projection (linear ops commute):
bilinear 2x with half-pixel centers == 0.25/0.75 stencil with edge
clamping, on DVE.
"""
import sys

for _p in ("/opt/pypackages", "/opt/trn_rl_repo"):
    if _p not in sys.path:
        sys.path.insert(0, _p)

import numpy as np

import concourse.bacc as bacc
import concourse.mybir as mybir
import concourse.tile as tile
from concourse.bass_utils import run_bass_kernel_spmd

F32 = mybir.dt.float32
F32R = mybir.dt.float32r
BF16 = mybir.dt.bfloat16
EXP = mybir.ActivationFunctionType.Exp
COPY = mybir.ActivationFunctionType.Copy

B, CX, CY, CM = 4, 256, 512, 64
H = W = 64
HY = WY = 32
N = H * W              # 4096 tokens
NH = N // 2            # 2048 query rows per core
NYC = HY * WY          # 1024 coarse tokens
KC = N // 128          # 32 key chunks
NU = 2 * KC            # 64 pipeline units (key chunk x row half)

_CACHE = {}


def _build(debug=False):
    nc = bacc.Bacc("TRN2", target_bir_lowering=False, debug=False,
                   enable_asserts=False)

    # ---- DRAM I/O (per-core layouts pre-arranged on host) ----
    # xs: [128, 8 * 1024] block-major: block b = [ch0-127 | ch128-255] of
    #     pixel columns b*512..(b+1)*512  (for fself over the full image)
    xs = nc.dram_tensor("xs", [128, 8192], F32R, kind="ExternalInput").ap()
    # xl: [128, 2 * 2048] ch-chunk-major: this core's 2048 query pixels
    xl = nc.dram_tensor("xl", [128, 4096], F32R, kind="ExternalInput").ap()
    # yb: [128, 4 * 1024] ch-chunk-major
    yb = nc.dram_tensor("yb", [128, 4096], F32R, kind="ExternalInput").ap()
    wpack = nc.dram_tensor("wpack", [128, 1093], F32R, kind="ExternalInput").ap()
    ones = nc.dram_tensor("ones", [1, 4096], F32R, kind="ExternalInput").ap()
    # out: [128, 2 * 2048] ch-chunk-major
    out = nc.dram_tensor("out", [128, 4096], F32, kind="ExternalOutput").ap()
    if debug:
        d_fy2 = nc.dram_tensor("d_fy2", [128, 4096], F32, kind="ExternalOutput").ap()
        d_fx2 = nc.dram_tensor("d_fx2", [128, 2048], F32, kind="ExternalOutput").ap()
        d_fself = nc.dram_tensor("d_fself", [128, 2080], F32, kind="ExternalOutput").ap()
        d_h1s = nc.dram_tensor("d_h1s", [65, 4096], F32, kind="ExternalOutput").ap()
        d_sim0 = nc.dram_tensor("d_sim0", [128, 1024], F32, kind="ExternalOutput").ap()
        d_fout = nc.dram_tensor("d_fout", [65, 2048], F32, kind="ExternalOutput").ap()
        d_scaled = nc.dram_tensor("d_scaled", [65, 2048], F32, kind="ExternalOutput").ap()

    with tile.TileContext(nc) as tc:
        with tc.tile_pool(name="sbW", bufs=1) as sbW, \
             tc.tile_pool(name="sbM", bufs=1) as sbM:
            # ---- long-lived SBUF ----
            t_xl = sbM.tile([128, 4096], F32R)     # fx input + residual
            fy2 = sbM.tile([128, 4096], F32R)      # upsampled fy, duplicated
            fx2 = sbM.tile([128, 2048], F32R)      # fx, duplicated
            fselfT = sbM.tile([128, 65 * KC], BF16)
            h1s_aug = sbM.tile([65, 4096], F32R)   # W_self1 @ x with ones row
            scaled = sbM.tile([65, 2048], F32R)    # [Z/Z; fout/Z] per row

            # ---- weights (single packed blob) ----
            t_wpack = sbW.tile([128, 1093], F32R)
            t_ws1t = t_wpack[:, 0:128]
            t_ws2a = t_wpack[0:65, 128:194]
            t_wx1t = t_wpack[:, 194:322]
            t_wx2t = t_wpack[0:64, 322:386]
            t_bx2 = t_wpack[0:64, 386:387].bitcast(F32)
            t_wy1t = t_wpack[:, 387:643]
            t_wy2t = t_wpack[0:64, 643:707]
            t_by2a = t_wpack[0:64, 707:709].bitcast(F32)
            t_wupt = t_wpack[0:65, 709:965]
            onecol = t_wpack[0:1, 965:1093].bitcast(F32)

            # ================= phase 1: projections =================
            sbP1_cm = tc.tile_pool(name="sbP1", bufs=1)
            sbP1 = sbP1_cm.__enter__()
            with tc.tile_pool(name="sbP2", bufs=1) as sbP2, \
                 tc.tile_pool(name="psP1", bufs=1, space="PSUM") as psP1:
                # input DMAs, critical-path first: weights, then y, then x
                nc.sync.dma_start(t_wpack[:], wpack[:])
                t_yb = sbP2.tile([128, 4096], F32R)
                nc.sync.dma_start(t_yb[:, 0:2048], yb[:, 0:2048])
                nc.sync.dma_start(t_yb[:, 2048:4096], yb[:, 2048:4096])
                nc.sync.dma_start(t_xl[:, 0:2048], xl[:, 0:2048])
                nc.sync.dma_start(t_xl[:, 2048:4096], xl[:, 2048:4096])
                nc.sync.dma_start(h1s_aug[64:65, :], ones[:, 0:4096])
                xs_tiles = []
                for blk in range(8):
                    t_xs = sbP1.tile([128, 1024], F32R, tag="xs", bufs=6,
                                     name=f"t_xs_{blk}")
                    xs_tiles.append(t_xs)
                    nc.sync.dma_start(t_xs[:], xs[:, blk * 1024:(blk + 1) * 1024])

                # warm the ACT exp table early
                t_dum = sbP1.tile([1, 32], F32)
                nc.vector.memset(t_dum[:], 0.0)
                t_dum2 = sbP1.tile([1, 32], F32)
                nc.scalar.activation(t_dum2[:], t_dum[:], EXP)

                # ---- fy path: h1y = Wy1 @ y ; then 0.75/0.25-scaled
                # biased copies (fyc75/fyc25), banded upsample ----
                h1y_s = sbP2.tile([64, 1024], F32R)
                fyc75 = sbP2.tile([64, 1024], F32)
                fyc25 = sbP2.tile([64, 1024], F32)
                for blk in range(2):
                    p = psP1.tile([64, 512], F32, tag="blk", bufs=4,
                                  name=f"p_h1y_{blk}")
                    for a in range(4):
                        nc.tensor.matmul(
                            p[:], t_wy1t[:, a * 64:(a + 1) * 64],
                            t_yb[:, blk * 2048 + a * 512:blk * 2048 + a * 512 + 512],
                            start=(a == 0), stop=(a == 3))
                    nc.scalar.activation(h1y_s[:, blk * 512:blk * 512 + 512],
                                         p[:], COPY)
                MUL, ADD_ = mybir.AluOpType.mult, mybir.AluOpType.add
                for blk in range(2):
                    p = psP1.tile([64, 512], F32, tag="blk", bufs=4,
                                  name=f"p_fyc_{blk}")
                    nc.tensor.matmul(p[:], t_wy2t,
                                     h1y_s[:, blk * 512:blk * 512 + 512],
                                     start=True, stop=True)
                    bs = slice(blk * 512, blk * 512 + 512)
                    nc.vector.tensor_scalar(fyc75[:, bs], p[:], 0.75,
                                            t_by2a[:, 0:1], MUL, ADD_)
                    nc.vector.tensor_scalar(fyc25[:, bs], p[:], 0.25,
                                            t_by2a[:, 1:2], MUL, ADD_)

                # H pass, 2 bands: [64, (32,32)] -> [64, (64,32)]
                fyH = sbP2.tile([64, 2048], F32)
                t1v = fyc75[:].rearrange("p (h w) -> p h w", h=32)
                t2v = fyc25[:].rearrange("p (h w) -> p h w", h=32)
                fe = fyH[:].rearrange("p (h two w) -> p h two w", h=32, two=2)
                # band 0 (needs fyc block 0 only: h 0..16 -> h' 0..30)
                nc.vector.tensor_add(fe[:, 0, 0, :], t1v[:, 0, :], t2v[:, 0, :])
                nc.vector.tensor_add(fe[:, 1:16, 0, :], t1v[:, 1:16, :], t2v[:, 0:15, :])
                nc.vector.tensor_add(fe[:, 0:15, 1, :], t1v[:, 0:15, :], t2v[:, 1:16, :])
                # band 1 (h' 31..63)
                nc.vector.tensor_add(fe[:, 16:32, 0, :], t1v[:, 16:32, :], t2v[:, 15:31, :])
                nc.vector.tensor_add(fe[:, 15:31, 1, :], t1v[:, 15:31, :], t2v[:, 16:32, :])
                nc.vector.tensor_add(fe[:, 31, 1, :], t1v[:, 31, :], t2v[:, 31, :])

                # scaled fyH copies, 2 bands: rows 0..30 / 31..63
                u1 = sbP2.tile([64, 2048], F32, tag="ut", bufs=2, name="u1")
                u2 = sbP2.tile([64, 2048], F32, tag="ut", bufs=2, name="u2")
                u1v = u1[:].rearrange("p (h w) -> p h w", h=64)
                u2v = u2[:].rearrange("p (h w) -> p h w", h=64)
                fyHv = fyH[:].rearrange("p (h w) -> p h w", h=64)
                nc.scalar.activation(u1[:, 0:31 * 32], fyH[:, 0:31 * 32],
                                     COPY, scale=0.75)
                nc.scalar.activation(u2[:, 0:31 * 32], fyH[:, 0:31 * 32],
                                     COPY, scale=0.25)
                nc.scalar.activation(u1[:, 31 * 32:2048], fyH[:, 31 * 32:2048],
                                     COPY, scale=0.75)
                nc.scalar.activation(u2[:, 31 * 32:2048], fyH[:, 31 * 32:2048],
                                     COPY, scale=0.25)

                # W pass + row-group duplication, 4 bands
                fw = fy2[0:64, :].rearrange("p (h w two) -> p h w two", h=64, two=2)
                for hs, c0, c1 in ((slice(0, 16), 0, 1024),
                                   (slice(16, 31), 1024, 1984),
                                   (slice(31, 48), 1984, 3072),
                                   (slice(48, 64), 3072, 4096)):
                    nc.vector.tensor_copy(fw[:, hs, 0, 0], fyHv[:, hs, 0])
                    nc.vector.tensor_add(fw[:, hs, 1:32, 0], u1v[:, hs, 1:32],
                                         u2v[:, hs, 0:31])
                    nc.vector.tensor_add(fw[:, hs, 0:31, 1], u1v[:, hs, 0:31],
                                         u2v[:, hs, 1:32])
                    nc.vector.tensor_copy(fw[:, hs, 31, 1], fyHv[:, hs, 31])
                    nc.vector.tensor_copy(fy2[64:128, c0:c1], fy2[0:64, c0:c1])

                # ---- fx path: h1x = Wx1 @ xl ; fx = Wx2 @ h1x + bx2 ----
                h1x_s = sbP1.tile([64, 2048], F32R)
                for blk in range(4):
                    p = psP1.tile([64, 512], F32, tag="blk", bufs=4,
                                  name=f"p_h1x_{blk}")
                    for a in range(2):
                        nc.tensor.matmul(
                            p[:], t_wx1t[:, a * 64:(a + 1) * 64],
                            t_xl[:, blk * 1024 + a * 512:blk * 1024 + a * 512 + 512],
                            start=(a == 0), stop=(a == 1))
                    nc.scalar.activation(h1x_s[:, blk * 512:blk * 512 + 512],
                                         p[:], COPY)
                for blk in range(4):
                    p = psP1.tile([64, 512], F32, tag="blk", bufs=4,
                                  name=f"p_fx_{blk}")
                    nc.tensor.matmul(p[:], t_wx2t,
                                     h1x_s[:, blk * 512:blk * 512 + 512],
                                     start=True, stop=True)
                    nc.vector.tensor_scalar_add(fx2[0:64, blk * 512:blk * 512 + 512],
                                                p[:], t_bx2)
                nc.vector.tensor_copy(fx2[64:128, :], fx2[0:64, :])

                # preamble of the fself stream (blocks/chunks 0-1) while the
                # attention pools are not yet open
                for blk in range(2):
                    p = psP1.tile([64, 512], F32, tag="blk", bufs=4,
                                  name=f"pp_h1s_{blk}")
                    for a in range(2):
                        nc.tensor.matmul(p[:], t_ws1t[:, a * 64:(a + 1) * 64],
                                         xs_tiles[blk][:, a * 512:(a + 1) * 512],
                                         start=(a == 0), stop=(a == 1))
                    nc.vector.tensor_copy(
                        h1s_aug[0:64, blk * 512:blk * 512 + 512], p[:])
                for ck in range(2):
                    p2 = psP1.tile([128, 66], F32, tag="blk", bufs=4,
                                   name=f"pp_fs_{ck}")
                    nc.tensor.matmul(p2[:], h1s_aug[:, ck * 128:(ck + 1) * 128],
                                     t_ws2a, start=True, stop=True)
                    nc.vector.tensor_copy(fselfT[:, ck * 65:(ck + 1) * 65],
                                          p2[:, 0:65])

                # ---- fself path: h1s = Ws1 @ xs ; fselfT chunks ----



            # ================= phase 2: attention (two half-loops) ====
            # half-loop h: all 32 key chunks for row half h (1024 rows).
            # fout accumulator per half = 2 PSUM banks, so half-loop 0 can
            # run a dedicated fself/h1s PSUM pool (2 banks) alongside the
            # sim pool (4 banks); half-loop 1 runs with both fout halves
            # live; the 1/Z half-0 tail overlaps half-loop 1 (GPSIMD
            # broadcast needs no PSUM).
            et_tiles = {}
            invzs = {}
            fout_accs = {}

            def sim_unit(pool, ck, h):
                ps = pool.tile([128, 1024], F32, tag="sim", bufs=2,
                               name=f"sim_{ck}_{h}")
                nc.tensor.matmul(
                    ps[:, 0:512], fy2[0:64, ck * 128:(ck + 1) * 128],
                    fx2[0:64, h * 1024:h * 1024 + 512],
                    start=True, stop=True)
                nc.tensor.matmul(
                    ps[:, 512:1024], fy2[64:128, ck * 128:(ck + 1) * 128],
                    fx2[64:128, h * 1024 + 512:h * 1024 + 1024],
                    start=True, stop=True)
                return ps

            def exp_unit(st, ck, h):
                et = sbM.tile([128, 1024], BF16, tag="et",
                              bufs=3 if debug else 4, name=f"et_{ck}_{h}")
                if debug and ck == 0 and h == 0:
                    d0 = sbM.tile([128, 1024], F32)
                    nc.vector.tensor_copy(d0[:], st[:])
                    nc.sync.dma_start(d_sim0[:], d0[:])
                nc.scalar.activation(et[:], st[:], EXP)
                return et

            def pv_unit(fout_acc, et, ck):
                w = fselfT[:, ck * 65:(ck + 1) * 65]
                nc.tensor.matmul(fout_acc[:, 0:512], w, et[:, 0:512],
                                 start=(ck == 0), stop=(ck == KC - 1))
                nc.tensor.matmul(fout_acc[:, 512:1024], w, et[:, 512:1024],
                                 start=(ck == 0), stop=(ck == KC - 1))

            def half_loop(h, psB, fs_hook, preamble=False):
                fout_acc = fout_accs[h]
                sims = {}
                sims[0] = sim_unit(psB, 0, h)
                sims[1] = sim_unit(psB, 1, h)
                sims[2] = sim_unit(psB, 2, h)
                if preamble:
                    fs_hook(-1)
                for ck in range(KC):
                    if fs_hook is not None:
                        fs_hook(ck)
                    et = exp_unit(sims.pop(ck), ck, h)
                    pv_unit(fout_acc, et, ck)
                    if ck + 3 < KC:
                        sims[ck + 3] = sim_unit(psB, ck + 3, h)

            def pre_tail(h):
                # 1/Z and fout scaling; no PSUM needed beyond fout_acc
                fout_acc = fout_accs[h]
                for s in range(2):
                    cs = slice(s * 512, (s + 1) * 512)
                    invz = sbM.tile([1, 512], F32, tag="zrow", bufs=2,
                                    name=f"invz_{h}_{s}")
                    nc.vector.reciprocal_approx_fast(invz[:], fout_acc[0:1, cs])
                    invzb = sbM.tile([128, 512], F32, tag="izb", bufs=2,
                                     name=f"invzb_{h}_{s}")
                    nc.gpsimd.partition_broadcast(invzb[:], invz[:])
                    nc.vector.tensor_mul(
                        scaled[:, h * 1024 + s * 512:h * 1024 + (s + 1) * 512],
                        fout_acc[:, cs], invzb[0:65, :])

            with tc.tile_pool(name="psA0", bufs=1, space="PSUM") as psA0:
                fout_accs[0] = psA0.tile([65, 1024], F32, name="fout0")
                with tc.tile_pool(name="psFS", bufs=1, space="PSUM") as psFS:

                    def fs_mms(ck):
                        p = psFS.tile([128, 66], F32, tag="fs", bufs=2,
                                      name=f"p_fs_{ck}")
                        nc.tensor.matmul(p[:],
                                         h1s_aug[:, ck * 128:(ck + 1) * 128],
                                         t_ws2a, start=True, stop=True)
                        nc.vector.tensor_copy(fselfT[:, ck * 65:(ck + 1) * 65],
                                              p[:, 0:65])

                    def h1s_mms(blk):
                        t_xs = xs_tiles[blk]
                        p = psFS.tile([64, 512], F32, tag="fs", bufs=2,
                                      name=f"p_h1s_{blk}")
                        for a in range(2):
                            nc.tensor.matmul(p[:],
                                             t_ws1t[:, a * 64:(a + 1) * 64],
                                             t_xs[:, a * 512:(a + 1) * 512],
                                             start=(a == 0), stop=(a == 1))
                        nc.vector.tensor_copy(
                            h1s_aug[0:64, blk * 512:blk * 512 + 512], p[:])

                    def fs_hook(ck):
                        if ck == -1:
                            return
                        if 2 * ck + 2 < 8:
                            h1s_mms(2 * ck + 2)
                            h1s_mms(2 * ck + 3)
                        if 2 * ck + 2 < KC:
                            fs_mms(2 * ck + 2)
                            fs_mms(2 * ck + 3)

                    with tc.tile_pool(name="psB0", bufs=1,
                                      space="PSUM") as psB0:
                        half_loop(0, psB0, fs_hook, preamble=True)

                with tc.tile_pool(name="psA1", bufs=1, space="PSUM") as psA1:
                    fout_accs[1] = psA1.tile([65, 1024], F32, name="fout1")
                    with tc.tile_pool(name="psB1", bufs=1,
                                      space="PSUM") as psB1:
                        pre_tail(0)
                        half_loop(1, psB1, None)
                    pre_tail(1)

                    if debug:
                        nc.sync.dma_start(d_fout[:, 0:1024], fout_accs[0][:])
                        nc.sync.dma_start(d_fout[:, 1024:2048], fout_accs[1][:])

                    # ====== final tail: up-projection + residual ======
                    with tc.tile_pool(name="psC", bufs=1, space="PSUM") as psC:
                        for q in range(4):
                            cs = slice(q * 512, (q + 1) * 512)
                            for a in range(2):
                                p = psC.tile([128, 512], F32, tag="up", bufs=4,
                                             name=f"p_up_{q}_{a}")
                                nc.tensor.matmul(
                                    p[:], t_wupt[:, a * 128:(a + 1) * 128],
                                    scaled[:, cs], start=True, stop=True)
                                out_s = sbM.tile([128, 512], F32, tag="tail",
                                                 bufs=4, name=f"out_s_{q}_{a}")
                                xlv = t_xl[:, q * 1024 + a * 512:
                                           q * 1024 + a * 512 + 512].bitcast(F32)
                                nc.vector.tensor_add(out_s[:], p[:], xlv)
                                nc.sync.dma_start(
                                    out[:, a * 2048 + q * 512:
                                        a * 2048 + (q + 1) * 512], out_s[:])

            sbP1_cm.__exit__(None, None, None)
            if debug:
                nc.sync.dma_start(d_fy2[:], fy2[:].bitcast(F32))
                nc.sync.dma_start(d_fx2[:], fx2[:].bitcast(F32))
                pass  # d_fself dump disabled (bf16)
                nc.sync.dma_start(d_h1s[:], h1s_aug[:].bitcast(F32))
                nc.sync.dma_start(d_scaled[:], scaled[:].bitcast(F32))

    nc.compile()
    return nc


def _prep_maps(x, y, W_self1, b_self1, W_self2, b_self2, W_x1, b_x1, W_x2,
               b_x2, W_y1, b_y1, W_y2, b_y2, W_up, b_up):
    f64 = np.float64

    def fold(W2, b1, b2):
        return (W2.astype(f64) @ b1.astype(f64) + b2.astype(f64)).astype(np.float32)

    ws2a = np.zeros((65, 66), np.float32)
    ws2a[64, 0] = 1.0
    ws2a[0:64, 1:65] = W_self2.T
    ws2a[64, 1:65] = fold(W_self2, b_self1, b_self2)
    bx2 = fold(W_x2, b_x1, b_x2).reshape(64, 1)
    _by2 = fold(W_y2, b_y1, b_y2).astype(np.float64)
    by2a = np.ascontiguousarray(
        np.stack([0.75 * _by2, 0.25 * _by2], axis=1).astype(np.float32))

    ws1t = np.ascontiguousarray(
        W_self1.T.reshape(2, 128, 64).transpose(1, 0, 2).reshape(128, 128))
    wx1t = np.ascontiguousarray(
        W_x1.T.reshape(2, 128, 64).transpose(1, 0, 2).reshape(128, 128))
    wy1t = np.ascontiguousarray(
        W_y1.T.reshape(4, 128, 64).transpose(1, 0, 2).reshape(128, 256))
    wx2t = np.ascontiguousarray(W_x2.T)
    wy2t = np.ascontiguousarray(W_y2.T)
    wupt = np.ascontiguousarray(
        np.concatenate([b_up.reshape(1, 256), W_up.T], axis=0))
    wp = np.zeros((128, 1093), np.float32)
    wp[:, 0:128] = ws1t
    wp[0:65, 128:194] = ws2a
    wp[:, 194:322] = wx1t
    wp[0:64, 322:386] = wx2t
    wp[0:64, 386:387] = bx2
    wp[:, 387:643] = wy1t
    wp[0:64, 643:707] = wy2t
    wp[0:64, 707:709] = by2a
    wp[0:65, 709:965] = wupt
    wp[0:1, 965:1093] = 1.0

    _ONES = np.ones((1, 4096), np.float32)
    maps = []
    for b in range(B):
        xf = x[b].reshape(CX, N)                                # [256, 4096]
        xs_h = np.ascontiguousarray(
            xf.reshape(2, 128, 8, 512).transpose(1, 2, 0, 3).reshape(128, 8192))
        yf = y[b].reshape(CY, NYC)
        yb_h = np.ascontiguousarray(
            yf.reshape(4, 128, 2, 512).transpose(1, 2, 0, 3).reshape(128, 4096))
        for half in range(2):
            xh = xf[:, half * NH:(half + 1) * NH]               # [256, 2048]
            xl_h = np.ascontiguousarray(
                xh.reshape(2, 128, 4, 512).transpose(1, 2, 0, 3).reshape(128, 4096))
            maps.append({
                "xs": xs_h, "xl": xl_h, "yb": yb_h,
                "wpack": wp, "ones": _ONES,
            })
    return maps


def _run(inputs, trace=False, trace_kwargs=None, debug=False):
    key = ("nc", debug)
    if key not in _CACHE:
        _CACHE[key] = _build(debug=debug)
    nc = _CACHE[key]
    maps = _prep_maps(**inputs)
    res = run_bass_kernel_spmd(nc, maps, list(range(8)), trace=trace,
                               **(trace_kwargs or {}))
    outs = np.empty((B, CX, H, W), np.float32)
    for b in range(B):
        for half in range(2):
            o = res.results[2 * b + half]["out"]                # [128, 4096]
            oh = o.reshape(128, 2, NH).transpose(1, 0, 2).reshape(CX, NH)
            outs[b, :, :, :].reshape(CX, N)[:, half * NH:(half + 1) * NH] = oh
    return outs, res


def kernel(**inputs):
    outs, _ = _run(inputs, trace=False)
    return outs



# revision 2
# speedup vs baseline: 1.2529x; 1.2529x over previous
"""Trainium2 Bass kernel for nn_BCA_17274358465235.

Module: out = x + conv1x1_up( softmax(fx @ fy_up^T) @ fself ) with
fx/fself = 2-layer 1x1-conv projections of x, fy = projection of
bilinearly-upsampled y.  B=4, CX=256, CY=512, CM=64, H=W=64 (N=4096
tokens), HY=WY=32.

Sharding: 8 cores = batch(4) x query-row-half(2).  Each core holds all
4096 keys (fy/fself replicated per batch) and 2048 query rows.  No
collectives.

Per-core algorithm (layouts chosen so no transposes are needed):
  all projection/sim matmuls run in fp16 (PE full rate + FWL); PV and
  exp outputs in bf16 (range), accumulation fp32 in PSUM.
  fself^T[key, c]  via second projection layer emitted transposed
  sim^T[key, row] = fy_f[:, keys]^T @ fx[:, rows]   (two key-chunks
      packed into PE row-groups 0-1 / 2-3 via duplicated operands)
  exp on ACT (no max-subtraction: |sim| < 70, fp32-safe)
  fout^T[c, row] += fself^T_chunk^T @ exp_chunk   (PSUM accumulation,
      ones-column in fself^T produces the softmax denominator Z free)
  out = x + W_up @ (fout^T * (1/Z)) + b_up   (b_up via ones-row in W_up)
The y-upsample runs after the channel projection (linear ops commute):
bilinear 2x with half-pixel centers == 0.25/0.75 stencil with edge
clamping, on DVE.  Second-layer weights are host-duplicated to
[64,128] so fx/fy come out of PSUM already replicated across both
partition halves (no separate duplication pass).
"""
import sys

for _p in ("/opt/pypackages", "/opt/trn_rl_repo"):
    if _p not in sys.path:
        sys.path.insert(0, _p)

import numpy as np

import concourse.bacc as bacc
import concourse.mybir as mybir
import concourse.tile as tile
from concourse.bass_utils import run_bass_kernel_spmd

F32 = mybir.dt.float32
F16 = mybir.dt.float16
BF16 = mybir.dt.bfloat16
EXP = mybir.ActivationFunctionType.Exp
COPY = mybir.ActivationFunctionType.Copy
MUL = mybir.AluOpType.mult
ADD_ = mybir.AluOpType.add

B, CX, CY, CM = 4, 256, 512, 64
H = W = 64
HY = WY = 32
N = H * W              # 4096 tokens
NH = N // 2            # 2048 query rows per core
NYC = HY * WY          # 1024 coarse tokens
KC = N // 128          # 32 key chunks

# wpack (f16) column offsets
_WS1, _WS2A, _WX1, _WX2, _WY1, _WY2, _WUP, _WEND = 0, 128, 194, 322, 450, 706, 834, 1090

_CACHE = {}


def _build(debug=False):
    nc = bacc.Bacc("TRN2", target_bir_lowering=False, debug=False,
                   enable_asserts=False)

    # ---- DRAM I/O (per-core layouts pre-arranged on host, all f16) ----
    # xs: [128, 8 * 1024] block-major: block b = [ch0-127 | ch128-255] of
    #     pixel columns b*512..(b+1)*512  (for fself over the full image)
    xs = nc.dram_tensor("xs", [128, 8192], F16, kind="ExternalInput").ap()
    # xl: [128, 4 * 1024] block-major: this core's 2048 query pixels
    xl = nc.dram_tensor("xl", [128, 4096], F16, kind="ExternalInput").ap()
    # yb: [128, 2 * 2048] block-major
    yb = nc.dram_tensor("yb", [128, 4096], F16, kind="ExternalInput").ap()
    wpack = nc.dram_tensor("wpack", [128, _WEND], F16, kind="ExternalInput").ap()
    bpack = nc.dram_tensor("bpack", [128, 3], F32, kind="ExternalInput").ap()
    ones = nc.dram_tensor("ones", [1, 4096], F16, kind="ExternalInput").ap()
    # out: [128, 2 * 2048] ch-chunk-major
    out = nc.dram_tensor("out", [128, 4096], F32, kind="ExternalOutput").ap()
    if debug:
        d_fy2 = nc.dram_tensor("d_fy2", [128, 4096], F16, kind="ExternalOutput").ap()
        d_fx2 = nc.dram_tensor("d_fx2", [128, 2048], F16, kind="ExternalOutput").ap()
        d_h1s = nc.dram_tensor("d_h1s", [65, 4096], F16, kind="ExternalOutput").ap()
        d_sim0 = nc.dram_tensor("d_sim0", [128, 1024], F32, kind="ExternalOutput").ap()
        d_fout = nc.dram_tensor("d_fout", [65, 2048], F32, kind="ExternalOutput").ap()
        d_scaled = nc.dram_tensor("d_scaled", [65, 2048], F16, kind="ExternalOutput").ap()

    with tile.TileContext(nc) as tc:
        with tc.tile_pool(name="sbW", bufs=1) as sbW, \
             tc.tile_pool(name="sbM", bufs=1) as sbM:
            # ---- long-lived SBUF ----
            t_xl = sbM.tile([128, 4096], F16)      # fx input + residual
            fy2 = sbM.tile([128, 4096], F16)       # upsampled fy, duplicated
            fx2 = sbM.tile([128, 2048], F16)       # fx, duplicated
            fselfT = sbM.tile([128, 65 * KC], BF16)
            h1s_aug = sbM.tile([65, 4096], F16)    # W_self1 @ x with ones row
            scaled = sbM.tile([65, 2048], F16)     # [Z/Z; fout/Z] per row

            # ---- weights (packed blobs) ----
            t_wpack = sbW.tile([128, _WEND], F16)
            t_bpack = sbW.tile([128, 3], F32)
            t_ws1t = t_wpack[:, _WS1:_WS1 + 128]
            t_ws2a = t_wpack[0:65, _WS2A:_WS2A + 66]
            t_wx1t = t_wpack[:, _WX1:_WX1 + 128]
            t_wx2t = t_wpack[0:64, _WX2:_WX2 + 128]
            t_wy1t = t_wpack[:, _WY1:_WY1 + 256]
            t_wy2t = t_wpack[0:64, _WY2:_WY2 + 128]
            t_wupt = t_wpack[0:65, _WUP:_WUP + 256]
            t_bx2 = t_bpack[:, 0:1]
            t_by75 = t_bpack[:, 1:2]
            t_by25 = t_bpack[:, 2:3]

            # ================= phase 1: projections =================
            sbP1_cm = tc.tile_pool(name="sbP1", bufs=1)
            sbP1 = sbP1_cm.__enter__()
            with tc.tile_pool(name="sbP2", bufs=1) as sbP2, \
                 tc.tile_pool(name="psP1", bufs=1, space="PSUM") as psP1:
                # input DMAs, critical-path first: weights, then y, then x
                nc.sync.dma_start(t_wpack[:], wpack[:])
                nc.sync.dma_start(t_bpack[:], bpack[:])
                t_yb = sbP2.tile([128, 4096], F16)
                nc.sync.dma_start(t_yb[:, 0:2048], yb[:, 0:2048])
                nc.sync.dma_start(t_yb[:, 2048:4096], yb[:, 2048:4096])
                nc.sync.dma_start(t_xl[:, 0:2048], xl[:, 0:2048])
                nc.sync.dma_start(t_xl[:, 2048:4096], xl[:, 2048:4096])
                nc.sync.dma_start(h1s_aug[64:65, :], ones[:, 0:4096])
                xs_tiles = []
                for blk in range(8):
                    t_xs = sbP1.tile([128, 1024], F16, tag="xs", bufs=8,
                                     name=f"t_xs_{blk}")
                    xs_tiles.append(t_xs)
                    nc.sync.dma_start(t_xs[:], xs[:, blk * 1024:(blk + 1) * 1024])

                # warm the ACT exp table early
                t_dum = sbP1.tile([1, 32], F32)
                nc.vector.memset(t_dum[:], 0.0)
                t_dum2 = sbP1.tile([1, 32], F32)
                nc.scalar.activation(t_dum2[:], t_dum[:], EXP)

                # ---- fy path: h1y = Wy1 @ y ; wy2(dup) -> fyc75/fyc25
                # (0.75/0.25-scaled biased copies on both partition
                # halves), banded upsample ----
                h1y_s = sbP2.tile([64, 1024], F16)
                fyc75 = sbP2.tile([128, 1024], F16)
                fyc25 = sbP2.tile([128, 1024], F16)
                for blk in range(2):
                    p = psP1.tile([64, 512], F32, tag="blk", bufs=4,
                                  name=f"p_h1y_{blk}")
                    for a in range(4):
                        nc.tensor.matmul(
                            p[:], t_wy1t[:, a * 64:(a + 1) * 64],
                            t_yb[:, blk * 2048 + a * 512:blk * 2048 + a * 512 + 512],
                            start=(a == 0), stop=(a == 3))
                    nc.scalar.activation(h1y_s[:, blk * 512:blk * 512 + 512],
                                         p[:], COPY)
                for blk in range(2):
                    p = psP1.tile([128, 512], F32, tag="blk", bufs=4,
                                  name=f"p_fyc_{blk}")
                    nc.tensor.matmul(p[:], t_wy2t,
                                     h1y_s[:, blk * 512:blk * 512 + 512],
                                     start=True, stop=True)
                    bs = slice(blk * 512, blk * 512 + 512)
                    nc.vector.tensor_scalar(fyc75[:, bs], p[:], 0.75,
                                            t_by75, MUL, ADD_)
                    nc.vector.tensor_scalar(fyc25[:, bs], p[:], 0.25,
                                            t_by25, MUL, ADD_)

                # H pass, 2 bands: [128, (32,32)] -> [128, (64,32)]
                fyH = sbP2.tile([128, 2048], F16)
                t1v = fyc75[:].rearrange("p (h w) -> p h w", h=32)
                t2v = fyc25[:].rearrange("p (h w) -> p h w", h=32)
                fe = fyH[:].rearrange("p (h two w) -> p h two w", h=32, two=2)
                # band 0 (needs fyc block 0 only: h 0..16 -> h' 0..30)
                nc.vector.tensor_add(fe[:, 0, 0, :], t1v[:, 0, :], t2v[:, 0, :])
                nc.vector.tensor_add(fe[:, 1:16, 0, :], t1v[:, 1:16, :], t2v[:, 0:15, :])
                nc.vector.tensor_add(fe[:, 0:15, 1, :], t1v[:, 0:15, :], t2v[:, 1:16, :])
                # band 1 (h' 31..63)
                nc.vector.tensor_add(fe[:, 16:32, 0, :], t1v[:, 16:32, :], t2v[:, 15:31, :])
                nc.vector.tensor_add(fe[:, 15:31, 1, :], t1v[:, 15:31, :], t2v[:, 16:32, :])
                nc.vector.tensor_add(fe[:, 31, 1, :], t1v[:, 31, :], t2v[:, 31, :])

                # 0.25-scaled fyH copy (ACT), 2 bands: rows 0..30 / 31..63
                u2 = sbP2.tile([128, 2048], F16)
                u2v = u2[:].rearrange("p (h w) -> p h w", h=64)
                fyHv = fyH[:].rearrange("p (h w) -> p h w", h=64)
                nc.scalar.activation(u2[:, 0:31 * 32], fyH[:, 0:31 * 32],
                                     COPY, scale=0.25)
                nc.scalar.activation(u2[:, 31 * 32:2048], fyH[:, 31 * 32:2048],
                                     COPY, scale=0.25)

                # W pass: fw = 0.75*fyH[w] + u2[w -+ 1], 2 bands
                fw = fy2[:].rearrange("p (h w two) -> p h w two", h=64, two=2)
                for hs in (slice(0, 31), slice(31, 64)):
                    nc.vector.tensor_copy(fw[:, hs, 0, 0], fyHv[:, hs, 0])
                    nc.vector.scalar_tensor_tensor(
                        fw[:, hs, 1:32, 0], fyHv[:, hs, 1:32], 0.75,
                        u2v[:, hs, 0:31], MUL, ADD_)
                    nc.vector.scalar_tensor_tensor(
                        fw[:, hs, 0:31, 1], fyHv[:, hs, 0:31], 0.75,
                        u2v[:, hs, 1:32], MUL, ADD_)
                    nc.vector.tensor_copy(fw[:, hs, 31, 1], fyHv[:, hs, 31])

                # ---- fx path: h1x = Wx1 @ xl ; fx = Wx2(dup) @ h1x + bx2 ----
                h1x_s = sbP2.tile([64, 2048], F16)
                for blk in range(4):
                    p = psP1.tile([64, 512], F32, tag="blk", bufs=4,
                                  name=f"p_h1x_{blk}")
                    for a in range(2):
                        nc.tensor.matmul(
                            p[:], t_wx1t[:, a * 64:(a + 1) * 64],
                            t_xl[:, blk * 1024 + a * 512:blk * 1024 + a * 512 + 512],
                            start=(a == 0), stop=(a == 1))
                    nc.scalar.activation(h1x_s[:, blk * 512:blk * 512 + 512],
                                         p[:], COPY)
                for blk in range(4):
                    p = psP1.tile([128, 512], F32, tag="blk", bufs=4,
                                  name=f"p_fx_{blk}")
                    nc.tensor.matmul(p[:], t_wx2t,
                                     h1x_s[:, blk * 512:blk * 512 + 512],
                                     start=True, stop=True)
                    nc.vector.tensor_scalar_add(fx2[:, blk * 512:blk * 512 + 512],
                                                p[:], t_bx2)

                # preamble of the fself stream (blocks/chunks 0-1) while the
                # attention pools are not yet open
                for blk in range(2):
                    p = psP1.tile([64, 512], F32, tag="blk", bufs=4,
                                  name=f"pp_h1s_{blk}")
                    for a in range(2):
                        nc.tensor.matmul(p[:], t_ws1t[:, a * 64:(a + 1) * 64],
                                         xs_tiles[blk][:, a * 512:(a + 1) * 512],
                                         start=(a == 0), stop=(a == 1))
                    nc.vector.tensor_copy(
                        h1s_aug[0:64, blk * 512:blk * 512 + 512], p[:])
                for ck in range(2):
                    p2 = psP1.tile([128, 66], F32, tag="blk", bufs=4,
                                   name=f"pp_fs_{ck}")
                    nc.tensor.matmul(p2[:], h1s_aug[:, ck * 128:(ck + 1) * 128],
                                     t_ws2a, start=True, stop=True)
                    nc.vector.tensor_copy(fselfT[:, ck * 65:(ck + 1) * 65],
                                          p2[:, 0:65])

            # ================= phase 2: attention (two half-loops) ====
            # half-loop h: all 32 key chunks for row half h (1024 rows).
            # fout accumulator per half = 2 PSUM banks, so half-loop 0 can
            # run a dedicated fself/h1s PSUM pool (2 banks) alongside the
            # sim pool (4 banks); half-loop 1 runs with both fout halves
            # live; the 1/Z half-0 tail overlaps half-loop 1 (GPSIMD
            # broadcast needs no PSUM).
            fout_accs = {}

            def sim_unit(pool, ck, h):
                ps = pool.tile([128, 1024], F32, tag="sim", bufs=2,
                               name=f"sim_{ck}_{h}")
                nc.tensor.matmul(
                    ps[:, 0:512], fy2[0:64, ck * 128:(ck + 1) * 128],
                    fx2[0:64, h * 1024:h * 1024 + 512],
                    start=True, stop=True)
                nc.tensor.matmul(
                    ps[:, 512:1024], fy2[64:128, ck * 128:(ck + 1) * 128],
                    fx2[64:128, h * 1024 + 512:h * 1024 + 1024],
                    start=True, stop=True)
                return ps

            def exp_unit(st, ck, h):
                et = sbM.tile([128, 1024], BF16, tag="et",
                              bufs=3 if debug else 4, name=f"et_{ck}_{h}")
                if debug and ck == 0 and h == 0:
                    d0 = sbM.tile([128, 1024], F32)
                    nc.vector.tensor_copy(d0[:], st[:])
                    nc.sync.dma_start(d_sim0[:], d0[:])
                nc.scalar.activation(et[:], st[:], EXP)
                return et

            def pv_unit(fout_acc, et, ck):
                w = fselfT[:, ck * 65:(ck + 1) * 65]
                nc.tensor.matmul(fout_acc[:, 0:512], w, et[:, 0:512],
                                 start=(ck == 0), stop=(ck == KC - 1))
                nc.tensor.matmul(fout_acc[:, 512:1024], w, et[:, 512:1024],
                                 start=(ck == 0), stop=(ck == KC - 1))

            def half_loop(h, psB, fs_hook, preamble=False):
                fout_acc = fout_accs[h]
                sims = {}
                sims[0] = sim_unit(psB, 0, h)
                sims[1] = sim_unit(psB, 1, h)
                sims[2] = sim_unit(psB, 2, h)
                if preamble:
                    fs_hook(-1)
                for ck in range(KC):
                    if fs_hook is not None:
                        fs_hook(ck)
                    et = exp_unit(sims.pop(ck), ck, h)
                    pv_unit(fout_acc, et, ck)
                    if ck + 3 < KC:
                        sims[ck + 3] = sim_unit(psB, ck + 3, h)

            def pre_tail(h):
                # 1/Z and fout scaling; no PSUM needed beyond fout_acc
                fout_acc = fout_accs[h]
                for s in range(2):
                    cs = slice(s * 512, (s + 1) * 512)
                    invz = sbM.tile([1, 512], F32, tag="zrow", bufs=2,
                                    name=f"invz_{h}_{s}")
                    nc.vector.reciprocal_approx_fast(invz[:], fout_acc[0:1, cs])
                    invzb = sbM.tile([128, 512], F32, tag="izb", bufs=2,
                                     name=f"invzb_{h}_{s}")
                    nc.gpsimd.partition_broadcast(invzb[:], invz[:])
                    nc.vector.tensor_mul(
                        scaled[:, h * 1024 + s * 512:h * 1024 + (s + 1) * 512],
                        fout_acc[:, cs], invzb[0:65, :])

            with tc.tile_pool(name="psA0", bufs=1, space="PSUM") as psA0:
                fout_accs[0] = psA0.tile([65, 1024], F32, name="fout0")
                with tc.tile_pool(name="psFS", bufs=1, space="PSUM") as psFS:

                    def fs_mms(ck):
                        p = psFS.tile([128, 66], F32, tag="fs", bufs=2,
                                      name=f"p_fs_{ck}")
                        nc.tensor.matmul(p[:],
                                         h1s_aug[:, ck * 128:(ck + 1) * 128],
                                         t_ws2a, start=True, stop=True)
                        nc.vector.tensor_copy(fselfT[:, ck * 65:(ck + 1) * 65],
                                              p[:, 0:65])

                    def h1s_mms(blk):
                        t_xs = xs_tiles[blk]
                        p = psFS.tile([64, 512], F32, tag="fs", bufs=2,
                                      name=f"p_h1s_{blk}")
                        for a in range(2):
                            nc.tensor.matmul(p[:],
                                             t_ws1t[:, a * 64:(a + 1) * 64],
                                             t_xs[:, a * 512:(a + 1) * 512],
                                             start=(a == 0), stop=(a == 1))
                        nc.vector.tensor_copy(
                            h1s_aug[0:64, blk * 512:blk * 512 + 512], p[:])

                    def fs_hook(ck):
                        if ck == -1:
                            return
                        if 2 * ck + 2 < 8:
                            h1s_mms(2 * ck + 2)
                            h1s_mms(2 * ck + 3)
                        if 2 * ck + 2 < KC:
                            fs_mms(2 * ck + 2)
                            fs_mms(2 * ck + 3)

                    with tc.tile_pool(name="psB0", bufs=1,
                                      space="PSUM") as psB0:
                        half_loop(0, psB0, fs_hook, preamble=True)

                with tc.tile_pool(name="psA1", bufs=1, space="PSUM") as psA1:
                    fout_accs[1] = psA1.tile([65, 1024], F32, name="fout1")
                    with tc.tile_pool(name="psB1", bufs=1,
                                      space="PSUM") as psB1:
                        pre_tail(0)
                        half_loop(1, psB1, None)
                    pre_tail(1)

                    if debug:
                        nc.sync.dma_start(d_fout[:, 0:1024], fout_accs[0][:])
                        nc.sync.dma_start(d_fout[:, 1024:2048], fout_accs[1][:])

                    # ====== final tail: up-projection + residual ======
                    with tc.tile_pool(name="psC", bufs=1, space="PSUM") as psC:
                        for q in range(4):
                            cs = slice(q * 512, (q + 1) * 512)
                            for a in range(2):
                                p = psC.tile([128, 512], F32, tag="up", bufs=4,
                                             name=f"p_up_{q}_{a}")
                                nc.tensor.matmul(
                                    p[:], t_wupt[:, a * 128:(a + 1) * 128],
                                    scaled[:, cs], start=True, stop=True)
                                out_s = sbM.tile([128, 512], F32, tag="tail",
                                                 bufs=4, name=f"out_s_{q}_{a}")
                                xlv = t_xl[:, q * 1024 + a * 512:
                                           q * 1024 + a * 512 + 512]
                                nc.vector.tensor_add(out_s[:], p[:], xlv)
                                nc.sync.dma_start(
                                    out[:, a * 2048 + q * 512:
                                        a * 2048 + (q + 1) * 512], out_s[:])

            sbP1_cm.__exit__(None, None, None)
            if debug:
                nc.sync.dma_start(d_fy2[:], fy2[:])
                nc.sync.dma_start(d_fx2[:], fx2[:])
                nc.sync.dma_start(d_h1s[:], h1s_aug[:])
                nc.sync.dma_start(d_scaled[:], scaled[:])

    nc.compile()
    return nc


def _prep_maps(x, y, W_self1, b_self1, W_self2, b_self2, W_x1, b_x1, W_x2,
               b_x2, W_y1, b_y1, W_y2, b_y2, W_up, b_up):
    f64 = np.float64
    f16 = np.float16

    def fold(W2, b1, b2):
        return (W2.astype(f64) @ b1.astype(f64) + b2.astype(f64)).astype(np.float32)

    ws2a = np.zeros((65, 66), np.float32)
    ws2a[64, 0] = 1.0
    ws2a[0:64, 1:65] = W_self2.T
    ws2a[64, 1:65] = fold(W_self2, b_self1, b_self2)
    bx2 = np.tile(fold(W_x2, b_x1, b_x2).reshape(64, 1), (2, 1))      # [128,1]
    _by2 = np.tile(fold(W_y2, b_y1, b_y2).astype(np.float64), 2)      # [128]
    by2a = np.ascontiguousarray(
        np.stack([0.75 * _by2, 0.25 * _by2], axis=1).astype(np.float32))

    ws1t = np.ascontiguousarray(
        W_self1.T.reshape(2, 128, 64).transpose(1, 0, 2).reshape(128, 128))
    wx1t = np.ascontiguousarray(
        W_x1.T.reshape(2, 128, 64).transpose(1, 0, 2).reshape(128, 128))
    wy1t = np.ascontiguousarray(
        W_y1.T.reshape(4, 128, 64).transpose(1, 0, 2).reshape(128, 256))
    wx2t = np.tile(np.ascontiguousarray(W_x2.T), (1, 2))              # [64,128]
    wy2t = np.tile(np.ascontiguousarray(W_y2.T), (1, 2))              # [64,128]
    wupt = np.ascontiguousarray(
        np.concatenate([b_up.reshape(1, 256), W_up.T], axis=0))
    wp = np.zeros((128, _WEND), f16)
    wp[:, _WS1:_WS1 + 128] = ws1t
    wp[0:65, _WS2A:_WS2A + 66] = ws2a
    wp[:, _WX1:_WX1 + 128] = wx1t
    wp[0:64, _WX2:_WX2 + 128] = wx2t
    wp[:, _WY1:_WY1 + 256] = wy1t
    wp[0:64, _WY2:_WY2 + 128] = wy2t
    wp[0:65, _WUP:_WUP + 256] = wupt
    bp = np.zeros((128, 3), np.float32)
    bp[:, 0:1] = bx2
    bp[:, 1:3] = by2a

    _ONES = np.ones((1, 4096), f16)
    maps = []
    for b in range(B):
        xf = x[b].reshape(CX, N)                                # [256, 4096]
        xs_h = np.ascontiguousarray(
            xf.reshape(2, 128, 8, 512).transpose(1, 2, 0, 3).reshape(128, 8192)
        ).astype(f16)
        yf = y[b].reshape(CY, NYC)
        yb_h = np.ascontiguousarray(
            yf.reshape(4, 128, 2, 512).transpose(1, 2, 0, 3).reshape(128, 4096)
        ).astype(f16)
        for half in range(2):
            xh = xf[:, half * NH:(half + 1) * NH]               # [256, 2048]
            xl_h = np.ascontiguousarray(
                xh.reshape(2, 128, 4, 512).transpose(1, 2, 0, 3).reshape(128, 4096)
            ).astype(f16)
            maps.append({
                "xs": xs_h, "xl": xl_h, "yb": yb_h,
                "wpack": wp, "bpack": bp, "ones": _ONES,
            })
    return maps


def _run(inputs, trace=False, trace_kwargs=None, debug=False):
    key = ("nc", debug)
    if key not in _CACHE:
        _CACHE[key] = _build(debug=debug)
    nc = _CACHE[key]
    maps = _prep_maps(**inputs)
    res = run_bass_kernel_spmd(nc, maps, list(range(8)), trace=trace,
                               **(trace_kwargs or {}))
    outs = np.empty((B, CX, H, W), np.float32)
    for b in range(B):
        for half in range(2):
            o = res.results[2 * b + half]["out"]                # [128, 4096]
            oh = o.reshape(128, 2, NH).transpose(1, 0, 2).reshape(CX, NH)
            outs[b, :, :, :].reshape(CX, N)[:, half * NH:(half + 1) * NH] = oh
    return outs, res


def kernel(**inputs):
    outs, _ = _run(inputs, trace=False)
    return outs


# revision 12
# speedup vs baseline: 1.2739x; 1.0167x over previous
"""Trainium2 Bass kernel for nn_BCA_17274358465235.

Module: out = x + conv1x1_up( softmax(fx @ fy_up^T) @ fself ) with
fx/fself = 2-layer 1x1-conv projections of x, fy = projection of
bilinearly-upsampled y.  B=4, CX=256, CY=512, CM=64, H=W=64 (N=4096
tokens), HY=WY=32.

Sharding: 8 cores = batch(4) x query-row-half(2).  Each core holds all
4096 keys (fy/fself replicated per batch) and 2048 query rows.  No
collectives.

Per-core algorithm (layouts chosen so no transposes are needed):
  fx / fy two-layer projections are folded on the host into single
  1x1 convs (associativity of 1x1 convs; folded in f64).  fself keeps
  the two-layer form because its second layer, emitted transposed,
  provides the [key, ch] layout plus a free all-ones Z column.
  All projection/sim matmuls run in fp16 (PE full rate + FWL); PV and
  exp outputs in bf16 (range), accumulation fp32 in PSUM.
  sim^T[key, row] = fy_f[:, keys]^T @ fx[:, rows]   (two key-chunks
      packed into PE row-groups 0-1 / 2-3 via duplicated operands)
  exp on ACT (no max-subtraction: |sim| < 70, fp32-safe)
  fout^T[c, row] += fself^T_chunk^T @ exp_chunk   (PSUM accumulation,
      ones-column in fself^T produces the softmax denominator Z free)
  out = x + W_up @ (fout^T * (1/Z)) + b_up   (b_up via ones-row in W_up)
The y-upsample runs after the channel projection (linear ops commute):
bilinear 2x with half-pixel centers == 0.25/0.75 stencil with edge
clamping, on DVE.  Keys are stored in even-w-then-odd-w order (host
permutes xs to match) so the W-pass writes are dense and 2x-capable.
Input DMAs are spread across the three DMA-capable queues; dummy
matmuls keep the PE's HAM clock warm through the projection phase.
"""
import sys

for _p in ("/opt/pypackages", "/opt/trn_rl_repo"):
    if _p not in sys.path:
        sys.path.insert(0, _p)

import numpy as np

import concourse.bacc as bacc
import concourse.mybir as mybir
import concourse.tile as tile
from concourse.bass_utils import run_bass_kernel_spmd

F32 = mybir.dt.float32
F16 = mybir.dt.float16
BF16 = mybir.dt.bfloat16
EXP = mybir.ActivationFunctionType.Exp
COPY = mybir.ActivationFunctionType.Copy
MUL = mybir.AluOpType.mult
ADD_ = mybir.AluOpType.add

B, CX, CY, CM = 4, 256, 512, 64
H = W = 64
HY = WY = 32
N = H * W              # 4096 tokens
NH = N // 2            # 2048 query rows per core
NYC = HY * WY          # 1024 coarse tokens
KC = N // 128          # 32 key chunks

# wpack (f16) column offsets (_WB: row 0 = [bx2 | by2] bias rows)
_WS1, _WS2A, _WXE, _WYE, _WUP, _WB, _WEND = 0, 128, 194, 450, 962, 1218, 1986

_CACHE = {}


def _build(debug=False):
    nc = bacc.Bacc("TRN2", target_bir_lowering=False, debug=False,
                   enable_asserts=False)

    # ---- DRAM I/O (per-core layouts pre-arranged on host, all f16) ----
    # xs: [128, 8 * 1024] block-major: block b = [ch0-127 | ch128-255] of
    #     key columns b*512..(b+1)*512 in even/odd-permuted key order
    xs = nc.dram_tensor("xs", [128, 8192], F16, kind="ExternalInput").ap()
    # xl: [128, 4 * 1024] block-major: this core's 2048 query pixels
    xl = nc.dram_tensor("xl", [128, 4096], F16, kind="ExternalInput").ap()
    # yb: [128, 2 * 2048] block-major
    yb = nc.dram_tensor("yb", [128, 4096], F16, kind="ExternalInput").ap()
    wpack = nc.dram_tensor("wpack", [128, _WEND], F16, kind="ExternalInput").ap()
    ones = nc.dram_tensor("ones", [1, 4096], F16, kind="ExternalInput").ap()
    # out: [128, 2 * 2048] ch-chunk-major
    out = nc.dram_tensor("out", [128, 4096], F32, kind="ExternalOutput").ap()
    if debug:
        d_fy2 = nc.dram_tensor("d_fy2", [128, 4096], F16, kind="ExternalOutput").ap()
        d_fx2 = nc.dram_tensor("d_fx2", [128, 2048], F16, kind="ExternalOutput").ap()
        d_h1s = nc.dram_tensor("d_h1s", [65, 4096], F16, kind="ExternalOutput").ap()
        d_sim0 = nc.dram_tensor("d_sim0", [128, 1024], F32, kind="ExternalOutput").ap()
        d_fout = nc.dram_tensor("d_fout", [65, 2048], F32, kind="ExternalOutput").ap()
        d_scaled = nc.dram_tensor("d_scaled", [65, 2048], F16, kind="ExternalOutput").ap()

    with tile.TileContext(nc) as tc:
        with tc.tile_pool(name="sbW", bufs=1) as sbW, \
             tc.tile_pool(name="sbM", bufs=1) as sbM:
            # ---- long-lived SBUF ----
            t_xl = sbM.tile([128, 4096], F16)      # fx input + residual
            fy2 = sbM.tile([128, 4096], F16)       # upsampled fy (even|odd)
            fx2 = sbM.tile([128, 2048], F16)       # fx, duplicated
            fselfT = sbM.tile([128, 65 * KC], BF16)
            h1s_aug = sbM.tile([65, 4096], F16)    # W_self1 @ x with ones row
            scaled = sbM.tile([65, 2048], F16)     # [Z/Z; fout/Z] per row

            # ---- weights (single packed blob) ----
            t_wpack = sbW.tile([128, _WEND], F16)
            t_ws1t = t_wpack[:, _WS1:_WS1 + 128]
            t_ws2a = t_wpack[0:65, _WS2A:_WS2A + 66]
            t_wxe = t_wpack[:, _WXE:_WXE + 256]
            t_wye = t_wpack[:, _WYE:_WYE + 512]
            t_wupt = t_wpack[0:65, _WUP:_WUP + 256]
            t_bx2r = t_wpack[0:1, _WB:_WB + 128]
            t_by2r = t_wpack[0:1, _WB + 128:_WB + 256]
            t_ones_r = t_wpack[0:1, _WB + 256:_WB + 768]

            # ================= phase 1: projections =================
            sbP1_cm = tc.tile_pool(name="sbP1", bufs=1)
            sbP1 = sbP1_cm.__enter__()
            with tc.tile_pool(name="sbP2", bufs=1) as sbP2, \
                 tc.tile_pool(name="psP1", bufs=1, space="PSUM") as psP1:
                t_yb = sbP2.tile([128, 4096], F16)
                xs_all = sbP1.tile([128, 8192], F16)
                xs_tiles = [xs_all[:, b * 1024:(b + 1) * 1024] for b in range(8)]

                # input DMAs spread across the DMA-capable queues
                nc.sync.dma_start(t_wpack[:], wpack[:])
                nc.scalar.dma_start(t_yb[:, 0:2048], yb[:, 0:2048])
                nc.scalar.dma_start(t_yb[:, 2048:4096], yb[:, 2048:4096])
                nc.gpsimd.dma_start(t_xl[:, 0:2048], xl[:, 0:2048])
                nc.gpsimd.dma_start(t_xl[:, 2048:4096], xl[:, 2048:4096])
                nc.gpsimd.dma_start(h1s_aug[64:65, :], ones[:, 0:4096])
                nc.sync.dma_start(xs_all[:, 0:4096], xs[:, 0:4096])
                nc.sync.dma_start(xs_all[:, 4096:8192], xs[:, 4096:8192])

                # warm the ACT exp table + the PE HAM clock early
                t_dum = sbP1.tile([1, 32], F32)
                t_warm = sbP1.tile([64, 512], F16)
                nc.vector.memset(t_warm[:], 0.25)
                nc.vector.memset(t_dum[:], 0.0)
                t_dum2 = sbP1.tile([1, 32], F32)
                nc.scalar.activation(t_dum2[:], t_dum[:], EXP)
                p_warm = psP1.tile([128, 512], F32, tag="warm", name="p_warm")
                for _w in range(12):
                    nc.tensor.matmul(p_warm[:], t_warm[:, 0:128], t_warm[:],
                                     start=True, stop=True)

                # ---- fy path: fyc = Wy_eff @ y + by2 (bias via a K=1
                # matmul against the ones row); 0.75/0.25 ACT copies ----
                ones_r = t_ones_r
                fyc75 = sbP2.tile([128, 1024], F16)
                fyc25 = sbP2.tile([128, 1024], F16)
                for blk in range(2):
                    p = psP1.tile([128, 512], F32, tag="blk", bufs=4,
                                  name=f"p_fyc_{blk}")
                    nc.tensor.matmul(p[:], t_by2r, ones_r,
                                     start=True, stop=False)
                    for a in range(4):
                        nc.tensor.matmul(
                            p[:], t_wye[:, a * 128:(a + 1) * 128],
                            t_yb[:, blk * 2048 + a * 512:blk * 2048 + a * 512 + 512],
                            start=False, stop=(a == 3))
                    bs = slice(blk * 512, blk * 512 + 512)
                    nc.scalar.activation(fyc75[:, bs], p[:], COPY, scale=0.75)
                    nc.scalar.activation(fyc25[:, bs], p[:], COPY, scale=0.25)

                # ---- fx path: fx = Wx_eff @ xl + bx2 ----
                fx_ps = []
                for blk in range(4):
                    p = psP1.tile([128, 512], F32, tag="blk", bufs=4,
                                  name=f"p_fx_{blk}")
                    nc.tensor.matmul(p[:], t_bx2r, ones_r,
                                     start=True, stop=False)
                    for a in range(2):
                        nc.tensor.matmul(
                            p[:], t_wxe[:, a * 128:(a + 1) * 128],
                            t_xl[:, blk * 1024 + a * 512:blk * 1024 + a * 512 + 512],
                            start=False, stop=(a == 1))
                    fx_ps.append(p)

                def fx_copy(blk):
                    nc.vector.tensor_copy(
                        fx2[:, blk * 512:blk * 512 + 512], fx_ps[blk][:])

                # H pass, 2 bands: [128, (32,32)] -> [128, (64,32)]
                fyH = sbP2.tile([128, 2048], F16)
                t1v = fyc75[:].rearrange("p (h w) -> p h w", h=32)
                t2v = fyc25[:].rearrange("p (h w) -> p h w", h=32)
                fe = fyH[:].rearrange("p (h two w) -> p h two w", h=32, two=2)
                # band 0 (needs fyc block 0 only: h 0..16 -> h' 0..30)
                nc.vector.tensor_add(fe[:, 0, 0, :], t1v[:, 0, :], t2v[:, 0, :])
                nc.vector.tensor_add(fe[:, 1:16, 0, :], t1v[:, 1:16, :], t2v[:, 0:15, :])
                nc.vector.tensor_add(fe[:, 0:15, 1, :], t1v[:, 0:15, :], t2v[:, 1:16, :])
                fx_copy(0)
                fx_copy(1)
                # band 1 (h' 31..63)
                nc.vector.tensor_add(fe[:, 16:32, 0, :], t1v[:, 16:32, :], t2v[:, 15:31, :])
                nc.vector.tensor_add(fe[:, 15:31, 1, :], t1v[:, 15:31, :], t2v[:, 16:32, :])
                nc.vector.tensor_add(fe[:, 31, 1, :], t1v[:, 31, :], t2v[:, 31, :])

                # 0.25-scaled fyH copy (ACT), 2 bands: rows 0..30 / 31..63
                u2 = sbP2.tile([128, 2048], F16)
                u2v = u2[:].rearrange("p (h w) -> p h w", h=64)
                fyHv = fyH[:].rearrange("p (h w) -> p h w", h=64)
                nc.scalar.activation(u2[:, 0:31 * 32], fyH[:, 0:31 * 32],
                                     COPY, scale=0.25)
                nc.scalar.activation(u2[:, 31 * 32:2048], fyH[:, 31 * 32:2048],
                                     COPY, scale=0.25)

                # W pass into even/odd key halves (dense stores):
                #   even w' = 0.75*fyH[w] + 0.25*fyH[w-1] (clamped)
                #   odd  w' = 0.75*fyH[w] + 0.25*fyH[w+1] (clamped)
                # band order = key-chunk consumption order (e0, e1, o0, o1)
                f2e = fy2[:, 0:2048].rearrange("p (h w) -> p h w", h=64)
                f2o = fy2[:, 2048:4096].rearrange("p (h w) -> p h w", h=64)
                for hs in (slice(0, 31), slice(31, 64)):
                    nc.vector.scalar_tensor_tensor(
                        f2e[:, hs, 1:32], fyHv[:, hs, 1:32], 0.75,
                        u2v[:, hs, 0:31], MUL, ADD_)
                    nc.vector.tensor_copy(f2e[:, hs, 0], fyHv[:, hs, 0])
                for hs in (slice(0, 31), slice(31, 64)):
                    nc.vector.scalar_tensor_tensor(
                        f2o[:, hs, 0:31], fyHv[:, hs, 0:31], 0.75,
                        u2v[:, hs, 1:32], MUL, ADD_)
                    nc.vector.tensor_copy(f2o[:, hs, 31], fyHv[:, hs, 31])

                # fx blocks 2-3 (only needed by half-loop 1)
                fx_copy(2)
                fx_copy(3)

                # preamble of the fself stream (blocks/chunks 0-1) while the
                # attention pools are not yet open
                for blk in range(2):
                    p = psP1.tile([64, 512], F32, tag="blk", bufs=4,
                                  name=f"pp_h1s_{blk}")
                    for a in range(2):
                        nc.tensor.matmul(p[:], t_ws1t[:, a * 64:(a + 1) * 64],
                                         xs_tiles[blk][:, a * 512:(a + 1) * 512],
                                         start=(a == 0), stop=(a == 1))
                    nc.vector.tensor_copy(
                        h1s_aug[0:64, blk * 512:blk * 512 + 512], p[:])
                for ck in range(2):
                    p2 = psP1.tile([128, 66], F32, tag="blk", bufs=4,
                                   name=f"pp_fs_{ck}")
                    nc.tensor.matmul(p2[:], h1s_aug[:, ck * 128:(ck + 1) * 128],
                                     t_ws2a, start=True, stop=True)
                    nc.vector.tensor_copy(fselfT[:, ck * 65:(ck + 1) * 65],
                                          p2[:, 0:65])

            # ================= phase 2: attention (two half-loops) ====
            fout_accs = {}

            def sim_unit(pool, ck, h):
                ps = pool.tile([128, 1024], F32, tag="sim", bufs=2,
                               name=f"sim_{ck}_{h}")
                nc.tensor.matmul(
                    ps[:, 0:512], fy2[0:64, ck * 128:(ck + 1) * 128],
                    fx2[0:64, h * 1024:h * 1024 + 512],
                    start=True, stop=True)
                nc.tensor.matmul(
                    ps[:, 512:1024], fy2[64:128, ck * 128:(ck + 1) * 128],
                    fx2[64:128, h * 1024 + 512:h * 1024 + 1024],
                    start=True, stop=True)
                return ps

            def exp_unit(st, ck, h):
                et = sbM.tile([128, 1024], BF16, tag="et",
                              bufs=3 if debug else 4, name=f"et_{ck}_{h}")
                if debug and ck == 0 and h == 0:
                    d0 = sbM.tile([128, 1024], F32)
                    nc.vector.tensor_copy(d0[:], st[:])
                    nc.sync.dma_start(d_sim0[:], d0[:])
                nc.scalar.activation(et[:], st[:], EXP)
                return et

            def pv_unit(fout_acc, et, ck):
                w = fselfT[:, ck * 65:(ck + 1) * 65]
                nc.tensor.matmul(fout_acc[:, 0:512], w, et[:, 0:512],
                                 start=(ck == 0), stop=(ck == KC - 1))
                nc.tensor.matmul(fout_acc[:, 512:1024], w, et[:, 512:1024],
                                 start=(ck == 0), stop=(ck == KC - 1))

            def half_loop(h, psB, fs_hook, preamble=False):
                fout_acc = fout_accs[h]
                sims = {}
                sims[0] = sim_unit(psB, 0, h)
                sims[1] = sim_unit(psB, 1, h)
                sims[2] = sim_unit(psB, 2, h)
                if preamble:
                    fs_hook(-1)
                for ck in range(KC):
                    if fs_hook is not None:
                        fs_hook(ck)
                    et = exp_unit(sims.pop(ck), ck, h)
                    pv_unit(fout_acc, et, ck)
                    if ck + 3 < KC:
                        sims[ck + 3] = sim_unit(psB, ck + 3, h)

            def pre_tail(h):
                # 1/Z and fout scaling; no PSUM needed beyond fout_acc.
                # recips first so the two gpsimd broadcasts run
                # back-to-back; then the two scale-muls.
                fout_acc = fout_accs[h]
                invzbs = []
                for s in range(2):
                    cs = slice(s * 512, (s + 1) * 512)
                    invz = sbM.tile([1, 512], F32, tag="zrow", bufs=2,
                                    name=f"invz_{h}_{s}")
                    nc.vector.reciprocal_approx_fast(invz[:], fout_acc[0:1, cs])
                    invzb = sbM.tile([128, 512], F32, tag="izb", bufs=2,
                                     name=f"invzb_{h}_{s}")
                    nc.gpsimd.partition_broadcast(invzb[:], invz[:])
                    invzbs.append(invzb)
                for s in range(2):
                    cs = slice(s * 512, (s + 1) * 512)
                    nc.vector.tensor_mul(
                        scaled[:, h * 1024 + s * 512:h * 1024 + (s + 1) * 512],
                        fout_acc[:, cs], invzbs[s][0:65, :])

            with tc.tile_pool(name="psA0", bufs=1, space="PSUM") as psA0:
                fout_accs[0] = psA0.tile([65, 1024], F32, name="fout0")
                with tc.tile_pool(name="psFS", bufs=1, space="PSUM") as psFS:

                    def fs_mms(ck):
                        p = psFS.tile([128, 66], F32, tag="fs", bufs=2,
                                      name=f"p_fs_{ck}")
                        nc.tensor.matmul(p[:],
                                         h1s_aug[:, ck * 128:(ck + 1) * 128],
                                         t_ws2a, start=True, stop=True)
                        nc.vector.tensor_copy(fselfT[:, ck * 65:(ck + 1) * 65],
                                              p[:, 0:65])

                    def h1s_mms(blk):
                        t_xs = xs_tiles[blk]
                        p = psFS.tile([64, 512], F32, tag="fs", bufs=2,
                                      name=f"p_h1s_{blk}")
                        for a in range(2):
                            nc.tensor.matmul(p[:],
                                             t_ws1t[:, a * 64:(a + 1) * 64],
                                             t_xs[:, a * 512:(a + 1) * 512],
                                             start=(a == 0), stop=(a == 1))
                        nc.vector.tensor_copy(
                            h1s_aug[0:64, blk * 512:blk * 512 + 512], p[:])

                    def fs_hook(ck):
                        if ck == -1:
                            return
                        if 2 * ck + 2 < 8:
                            h1s_mms(2 * ck + 2)
                            h1s_mms(2 * ck + 3)
                        if 2 * ck + 2 < KC:
                            fs_mms(2 * ck + 2)
                            fs_mms(2 * ck + 3)

                    with tc.tile_pool(name="psB0", bufs=1,
                                      space="PSUM") as psB0:
                        half_loop(0, psB0, fs_hook, preamble=True)

                with tc.tile_pool(name="psA1", bufs=1, space="PSUM") as psA1:
                    fout_accs[1] = psA1.tile([65, 1024], F32, name="fout1")
                    with tc.tile_pool(name="psB1", bufs=1,
                                      space="PSUM") as psB1:
                        pre_tail(0)
                        half_loop(1, psB1, None)
                    pre_tail(1)

                    if debug:
                        nc.sync.dma_start(d_fout[:, 0:1024], fout_accs[0][:])
                        nc.sync.dma_start(d_fout[:, 1024:2048], fout_accs[1][:])

                    # ====== final tail: up-projection + residual ======
                    dma_engines = [nc.sync, nc.scalar]
                    with tc.tile_pool(name="psC", bufs=1, space="PSUM") as psC:
                        for q in range(4):
                            cs = slice(q * 512, (q + 1) * 512)
                            for a in range(2):
                                p = psC.tile([128, 512], F32, tag="up", bufs=4,
                                             name=f"p_up_{q}_{a}")
                                nc.tensor.matmul(
                                    p[:], t_wupt[:, a * 128:(a + 1) * 128],
                                    scaled[:, cs], start=True, stop=True)
                                out_s = sbM.tile([128, 512], F32, tag="tail",
                                                 bufs=8, name=f"out_s_{q}_{a}")
                                xlv = t_xl[:, q * 1024 + a * 512:
                                           q * 1024 + a * 512 + 512]
                                nc.vector.tensor_add(out_s[:], p[:], xlv)
                                dma_engines[(2 * q + a) % 2].dma_start(
                                    out[:, a * 2048 + q * 512:
                                        a * 2048 + (q + 1) * 512], out_s[:])

            sbP1_cm.__exit__(None, None, None)
            if debug:
                nc.sync.dma_start(d_fy2[:], fy2[:])
                nc.sync.dma_start(d_fx2[:], fx2[:])
                nc.sync.dma_start(d_h1s[:], h1s_aug[:])
                nc.sync.dma_start(d_scaled[:], scaled[:])

    nc.compile()
    return nc


def _key_perm():
    """Key order: all even-w pixels (h-major), then all odd-w pixels."""
    k = np.arange(2048)
    even = (k // 32) * 64 + (k % 32) * 2
    odd = even + 1
    return np.concatenate([even, odd])


def _prep_maps(x, y, W_self1, b_self1, W_self2, b_self2, W_x1, b_x1, W_x2,
               b_x2, W_y1, b_y1, W_y2, b_y2, W_up, b_up):
    f64 = np.float64
    f16 = np.float16

    def fold(W2, b1, b2):
        return (W2.astype(f64) @ b1.astype(f64) + b2.astype(f64)).astype(np.float32)

    ws2a = np.zeros((65, 66), np.float32)
    ws2a[64, 0] = 1.0
    ws2a[0:64, 1:65] = W_self2.T
    ws2a[64, 1:65] = fold(W_self2, b_self1, b_self2)

    ws1t = np.ascontiguousarray(
        W_self1.T.reshape(2, 128, 64).transpose(1, 0, 2).reshape(128, 128))
    # fused single-layer fx / fy weights (f64 fold), K-chunked and
    # column-duplicated: [128, nchunk*128]
    wxe = (W_x2.astype(f64) @ W_x1.astype(f64)).T      # [256, 64]
    wxe = np.concatenate(
        [np.tile(wxe.reshape(2, 128, 64)[a], (1, 2)) for a in range(2)], 1)
    wye = (W_y2.astype(f64) @ W_y1.astype(f64)).T      # [512, 64]
    wye = np.concatenate(
        [np.tile(wye.reshape(4, 128, 64)[a], (1, 2)) for a in range(4)], 1)
    wupt = np.ascontiguousarray(
        np.concatenate([b_up.reshape(1, 256), W_up.T], axis=0))
    wp = np.zeros((128, _WEND), f16)
    wp[:, _WS1:_WS1 + 128] = ws1t
    wp[0:65, _WS2A:_WS2A + 66] = ws2a
    wp[:, _WXE:_WXE + 256] = wxe
    wp[:, _WYE:_WYE + 512] = wye
    wp[0:65, _WUP:_WUP + 256] = wupt
    wp[0, _WB:_WB + 128] = np.tile(fold(W_x2, b_x1, b_x2), 2)
    wp[0, _WB + 128:_WB + 256] = np.tile(fold(W_y2, b_y1, b_y2), 2)
    wp[0, _WB + 256:_WB + 768] = 1.0

    _ONES = np.ones((1, 4096), f16)
    perm = _key_perm()
    maps = []
    for b in range(B):
        xf = x[b].reshape(CX, N)                                # [256, 4096]
        xs_h = np.ascontiguousarray(
            xf[:, perm].reshape(2, 128, 8, 512).transpose(1, 2, 0, 3)
            .reshape(128, 8192)).astype(f16)
        yf = y[b].reshape(CY, NYC)
        yb_h = np.ascontiguousarray(
            yf.reshape(4, 128, 2, 512).transpose(1, 2, 0, 3).reshape(128, 4096)
        ).astype(f16)
        for half in range(2):
            xh = xf[:, half * NH:(half + 1) * NH]               # [256, 2048]
            xl_h = np.ascontiguousarray(
                xh.reshape(2, 128, 4, 512).transpose(1, 2, 0, 3).reshape(128, 4096)
            ).astype(f16)
            maps.append({
                "xs": xs_h, "xl": xl_h, "yb": yb_h,
                "wpack": wp, "ones": _ONES,
            })
    return maps


def _run(inputs, trace=False, trace_kwargs=None, debug=False):
    key = ("nc", debug)
    if key not in _CACHE:
        _CACHE[key] = _build(debug=debug)
    nc = _CACHE[key]
    maps = _prep_maps(**inputs)
    res = run_bass_kernel_spmd(nc, maps, list(range(8)), trace=trace,
                               **(trace_kwargs or {}))
    outs = np.empty((B, CX, H, W), np.float32)
    for b in range(B):
        for half in range(2):
            o = res.results[2 * b + half]["out"]                # [128, 4096]
            oh = o.reshape(128, 2, NH).transpose(1, 0, 2).reshape(CX, NH)
            outs[b, :, :, :].reshape(CX, N)[:, half * NH:(half + 1) * NH] = oh
    return outs, res


def kernel(**inputs):
    outs, _ = _run(inputs, trace=False)
    return outs


# revision 13
# speedup vs baseline: 1.3702x; 1.0756x over previous
"""Trainium2 Bass kernel for nn_BCA_17274358465235.

Module: out = x + conv1x1_up( softmax(fx @ fy_up^T) @ fself ) with
fx/fself = 2-layer 1x1-conv projections of x, fy = projection of
bilinearly-upsampled y.  B=4, CX=256, CY=512, CM=64, H=W=64 (N=4096
tokens), HY=WY=32.

Sharding: 8 cores = batch(4) x query-row-half(2).  Each core holds all
4096 keys (fy/fself replicated per batch) and 2048 query rows.  No
collectives.

Per-core algorithm (layouts chosen so no transposes are needed):
  fx / fy two-layer projections are folded on the host into single
  1x1 convs (associativity of 1x1 convs; folded in f64).  fself keeps
  the two-layer form because its second layer, emitted transposed,
  provides the [key, ch] layout plus a free all-ones Z column.
  All projection/sim matmuls run in fp16 (PE full rate + FWL); PV and
  exp outputs in bf16 (range), accumulation fp32 in PSUM.
  sim^T[key, row] = fy_f[:, keys]^T @ fx[:, rows]   (two key-chunks
      packed into PE row-groups 0-1 / 2-3 via duplicated operands)
  exp on ACT (no max-subtraction: |sim| < 70, fp32-safe)
  fout^T[c, row] += fself^T_chunk^T @ exp_chunk   (PSUM accumulation,
      ones-column in fself^T produces the softmax denominator Z free)
  out = x + W_up @ (fout^T * (1/Z)) + b_up   (b_up via ones-row in W_up)
The y-upsample runs after the channel projection (linear ops commute):
bilinear 2x with half-pixel centers == 0.25/0.75 stencil with edge
clamping, on DVE.  Keys are stored in even-w-then-odd-w order (host
permutes xs to match) so the W-pass writes are dense and 2x-capable.
Input DMAs are spread across the three DMA-capable queues; dummy
matmuls keep the PE's HAM clock warm through the projection phase.
"""
import sys

for _p in ("/opt/pypackages", "/opt/trn_rl_repo"):
    if _p not in sys.path:
        sys.path.insert(0, _p)

import numpy as np

import concourse.bacc as bacc
import concourse.mybir as mybir
import concourse.tile as tile
from concourse.bass_utils import run_bass_kernel_spmd

F32 = mybir.dt.float32
F16 = mybir.dt.float16
BF16 = mybir.dt.bfloat16
EXP = mybir.ActivationFunctionType.Exp
COPY = mybir.ActivationFunctionType.Copy
MUL = mybir.AluOpType.mult
ADD_ = mybir.AluOpType.add

B, CX, CY, CM = 4, 256, 512, 64
H = W = 64
HY = WY = 32
N = H * W              # 4096 tokens
NH = N // 2            # 2048 query rows per core
NYC = HY * WY          # 1024 coarse tokens
KC = N // 128          # 32 key chunks

# wpack (f16) column offsets (_WB: row 0 = [bx2 | by2] bias rows)
_WS1, _WS2A, _WXE, _WYE, _WUP, _WB, _WEND = 0, 128, 194, 450, 962, 1218, 1986

_CACHE = {}


def _build(debug=False):
    nc = bacc.Bacc("TRN2", target_bir_lowering=False, debug=False,
                   enable_asserts=False)

    # ---- DRAM I/O (per-core layouts pre-arranged on host, all f16) ----
    # xs: [128, 8 * 1024] block-major: block b = [ch0-127 | ch128-255] of
    #     key columns b*512..(b+1)*512 in even/odd-permuted key order
    xs = nc.dram_tensor("xs", [128, 8192], F16, kind="ExternalInput").ap()
    # xl: [128, 4 * 1024] block-major: this core's 2048 query pixels
    xl = nc.dram_tensor("xl", [128, 4096], F16, kind="ExternalInput").ap()
    # yb: [128, 2 * 2048] block-major
    yb = nc.dram_tensor("yb", [128, 4096], F16, kind="ExternalInput").ap()
    wpack = nc.dram_tensor("wpack", [128, _WEND], F16, kind="ExternalInput").ap()
    ones = nc.dram_tensor("ones", [1, 4096], F16, kind="ExternalInput").ap()
    # out: [128, 2 * 2048] ch-chunk-major
    out = nc.dram_tensor("out", [128, 4096], F32, kind="ExternalOutput").ap()
    if debug:
        d_fy2 = nc.dram_tensor("d_fy2", [128, 4096], F16, kind="ExternalOutput").ap()
        d_fx2 = nc.dram_tensor("d_fx2", [128, 2048], F16, kind="ExternalOutput").ap()
        d_h1s = nc.dram_tensor("d_h1s", [65, 4096], F16, kind="ExternalOutput").ap()
        d_sim0 = nc.dram_tensor("d_sim0", [128, 1024], F32, kind="ExternalOutput").ap()
        d_fout = nc.dram_tensor("d_fout", [65, 2048], F32, kind="ExternalOutput").ap()
        d_scaled = nc.dram_tensor("d_scaled", [65, 2048], F16, kind="ExternalOutput").ap()

    with tile.TileContext(nc) as tc:
        with tc.tile_pool(name="sbW", bufs=1) as sbW, \
             tc.tile_pool(name="sbM", bufs=1) as sbM:
            # ---- long-lived SBUF ----
            t_xl = sbM.tile([128, 4096], F16)      # fx input + residual
            fy2 = sbM.tile([128, 4096], F16)       # upsampled fy (even|odd)
            fx2 = sbM.tile([128, 2048], F16)       # fx, duplicated
            fselfT = sbM.tile([128, 65 * KC], BF16)
            h1s_aug = sbM.tile([65, 4096], F16)    # W_self1 @ x with ones row
            scaled = sbM.tile([65, 2048], F16)     # [Z/Z; fout/Z] per row

            # ---- weights (single packed blob) ----
            t_wpack = sbW.tile([128, _WEND], F16)
            t_ws1t = t_wpack[:, _WS1:_WS1 + 128]
            t_ws2a = t_wpack[0:65, _WS2A:_WS2A + 66]
            t_wxe = t_wpack[:, _WXE:_WXE + 256]
            t_wye = t_wpack[:, _WYE:_WYE + 512]
            t_wupt = t_wpack[0:65, _WUP:_WUP + 256]
            t_bx2r = t_wpack[0:1, _WB:_WB + 128]
            t_by2r = t_wpack[0:1, _WB + 128:_WB + 256]
            t_ones_r = t_wpack[0:1, _WB + 256:_WB + 768]

            # ================= phase 1: projections =================
            sbP1_cm = tc.tile_pool(name="sbP1", bufs=1)
            sbP1 = sbP1_cm.__enter__()
            with tc.tile_pool(name="sbP2", bufs=1) as sbP2, \
                 tc.tile_pool(name="psP1", bufs=1, space="PSUM") as psP1:
                t_yb = sbP2.tile([128, 4096], F16)
                xs_all = sbP1.tile([128, 8192], F16)
                xs_tiles = [xs_all[:, b * 1024:(b + 1) * 1024] for b in range(8)]

                # input DMAs: one queue, hand-ordered by need time.
                # exp0 needs only yb half 0 (fy band 0) and xl half 0
                # (fx blocks 0-1); xs/xl1 trickle in under the marathon.
                nc.sync.dma_start(t_wpack[:], wpack[:])
                nc.sync.dma_start(t_yb[:, 0:2048], yb[:, 0:2048])
                nc.sync.dma_start(t_xl[:, 0:2048], xl[:, 0:2048])
                nc.sync.dma_start(t_yb[:, 2048:4096], yb[:, 2048:4096])
                nc.sync.dma_start(xs_all[:, 0:2048], xs[:, 0:2048])
                nc.sync.dma_start(t_xl[:, 2048:4096], xl[:, 2048:4096])
                nc.sync.dma_start(xs_all[:, 2048:4096], xs[:, 2048:4096])
                nc.sync.dma_start(xs_all[:, 4096:6144], xs[:, 4096:6144])
                nc.sync.dma_start(xs_all[:, 6144:8192], xs[:, 6144:8192])
                nc.gpsimd.dma_start(h1s_aug[64:65, :], ones[:, 0:4096])

                # warm the ACT exp table + the PE HAM clock early
                t_dum = sbP1.tile([1, 32], F32)
                t_warm = sbP1.tile([64, 512], F16)
                nc.vector.memset(t_warm[:], 0.25)
                nc.vector.memset(t_dum[:], 0.0)
                t_dum2 = sbP1.tile([1, 32], F32)
                nc.scalar.activation(t_dum2[:], t_dum[:], EXP)
                p_warm = psP1.tile([128, 512], F32, tag="warm", name="p_warm")
                for _w in range(6):
                    nc.tensor.matmul(p_warm[:], t_warm[:, 0:128], t_warm[:],
                                     start=True, stop=True)

                # ---- fy/fx matmuls (PE) in DMA-arrival order ----
                ones_r = t_ones_r
                fyc75 = sbP2.tile([128, 1024], F16)
                fyc25 = sbP2.tile([128, 1024], F16)
                fyc_ps = []
                for blk in range(2):
                    p = psP1.tile([128, 512], F32, tag="blk", bufs=4,
                                  name=f"p_fyc_{blk}")
                    nc.tensor.matmul(p[:], t_by2r, ones_r,
                                     start=True, stop=False)
                    for a in range(4):
                        nc.tensor.matmul(
                            p[:], t_wye[:, a * 128:(a + 1) * 128],
                            t_yb[:, blk * 2048 + a * 512:blk * 2048 + a * 512 + 512],
                            start=False, stop=(a == 3))
                    fyc_ps.append(p)
                    if blk == 0:
                        # fx blocks 0-1 (xl half 0) right behind yb half 0
                        fx_ps = []
                        for fblk in range(2):
                            pf = psP1.tile([128, 512], F32, tag="blk", bufs=4,
                                           name=f"p_fx_{fblk}")
                            nc.tensor.matmul(pf[:], t_bx2r, ones_r,
                                             start=True, stop=False)
                            for a in range(2):
                                nc.tensor.matmul(
                                    pf[:], t_wxe[:, a * 128:(a + 1) * 128],
                                    t_xl[:, fblk * 1024 + a * 512:
                                         fblk * 1024 + a * 512 + 512],
                                    start=False, stop=(a == 1))
                            fx_ps.append(pf)

                def fyc_copy(blk):
                    bs = slice(blk * 512, blk * 512 + 512)
                    nc.vector.tensor_scalar_mul(fyc75[:, bs], fyc_ps[blk][:], 0.75)
                    nc.vector.tensor_scalar_mul(fyc25[:, bs], fyc_ps[blk][:], 0.25)

                def fx_copy(blk):
                    nc.vector.tensor_copy(
                        fx2[:, blk * 512:blk * 512 + 512], fx_ps[blk][:])

                # ---- upsample chain (DVE only; ACT stays free for exps).
                # band 0 first (gates exp0), band 1 after yb half 1. ----
                fyH = sbP2.tile([128, 2048], F16)
                u2 = sbP2.tile([128, 2048], F16)
                t1v = fyc75[:].rearrange("p (h w) -> p h w", h=32)
                t2v = fyc25[:].rearrange("p (h w) -> p h w", h=32)
                fe = fyH[:].rearrange("p (h two w) -> p h two w", h=32, two=2)
                u2v = u2[:].rearrange("p (h w) -> p h w", h=64)
                fyHv = fyH[:].rearrange("p (h w) -> p h w", h=64)
                f2e = fy2[:, 0:2048].rearrange("p (h w) -> p h w", h=64)
                f2o = fy2[:, 2048:4096].rearrange("p (h w) -> p h w", h=64)

                def w_pass(which, hs):
                    if which == 0:
                        nc.vector.scalar_tensor_tensor(
                            f2e[:, hs, 1:32], fyHv[:, hs, 1:32], 0.75,
                            u2v[:, hs, 0:31], MUL, ADD_)
                        nc.vector.tensor_copy(f2e[:, hs, 0], fyHv[:, hs, 0])
                    else:
                        nc.vector.scalar_tensor_tensor(
                            f2o[:, hs, 0:31], fyHv[:, hs, 0:31], 0.75,
                            u2v[:, hs, 1:32], MUL, ADD_)
                        nc.vector.tensor_copy(f2o[:, hs, 31], fyHv[:, hs, 31])

                b0, b1 = slice(0, 31), slice(31, 64)
                # band 0: H pass h' 0..30, u2 rows 0..30, W pass even
                fyc_copy(0)
                nc.vector.tensor_add(fe[:, 0, 0, :], t1v[:, 0, :], t2v[:, 0, :])
                nc.vector.tensor_add(fe[:, 1:16, 0, :], t1v[:, 1:16, :], t2v[:, 0:15, :])
                nc.vector.tensor_add(fe[:, 0:15, 1, :], t1v[:, 0:15, :], t2v[:, 1:16, :])
                fx_copy(0)
                fx_copy(1)
                nc.vector.tensor_scalar_mul(u2[:, 0:31 * 32], fyH[:, 0:31 * 32], 0.25)
                w_pass(0, b0)
                # band 1 (yb half 1): H pass h' 31..63, u2, W passes
                fyc_copy(1)
                nc.vector.tensor_add(fe[:, 16:32, 0, :], t1v[:, 16:32, :], t2v[:, 15:31, :])
                nc.vector.tensor_add(fe[:, 15:31, 1, :], t1v[:, 15:31, :], t2v[:, 16:32, :])
                nc.vector.tensor_add(fe[:, 31, 1, :], t1v[:, 31, :], t2v[:, 31, :])
                nc.vector.tensor_scalar_mul(u2[:, 31 * 32:2048],
                                            fyH[:, 31 * 32:2048], 0.25)
                w_pass(0, b1)
                w_pass(1, b0)
                w_pass(1, b1)


            # ================= phase 2: attention (two half-loops) ====
            fout_accs = {}

            def sim_unit(pool, ck, h):
                ps = pool.tile([128, 1024], F32, tag="sim", bufs=2,
                               name=f"sim_{ck}_{h}")
                nc.tensor.matmul(
                    ps[:, 0:512], fy2[0:64, ck * 128:(ck + 1) * 128],
                    fx2[0:64, h * 1024:h * 1024 + 512],
                    start=True, stop=True)
                nc.tensor.matmul(
                    ps[:, 512:1024], fy2[64:128, ck * 128:(ck + 1) * 128],
                    fx2[64:128, h * 1024 + 512:h * 1024 + 1024],
                    start=True, stop=True)
                return ps

            def exp_unit(st, ck, h):
                et = sbM.tile([128, 1024], BF16, tag="et",
                              bufs=3 if debug else 6, name=f"et_{ck}_{h}")
                if debug and ck == 0 and h == 0:
                    d0 = sbM.tile([128, 1024], F32)
                    nc.vector.tensor_copy(d0[:], st[:])
                    nc.sync.dma_start(d_sim0[:], d0[:])
                nc.scalar.activation(et[:], st[:], EXP)
                return et

            def pv_unit(fout_acc, et, ck):
                w = fselfT[:, ck * 65:(ck + 1) * 65]
                nc.tensor.matmul(fout_acc[:, 0:512], w, et[:, 0:512],
                                 start=(ck == 0), stop=(ck == KC - 1))
                nc.tensor.matmul(fout_acc[:, 512:1024], w, et[:, 512:1024],
                                 start=(ck == 0), stop=(ck == KC - 1))

            def half_loop(h, psB, fs_hook, preamble=False):
                fout_acc = fout_accs[h]
                sims = {}
                sims[0] = sim_unit(psB, 0, h)
                sims[1] = sim_unit(psB, 1, h)
                sims[2] = sim_unit(psB, 2, h)
                if preamble:
                    fs_hook(-1)
                for ck in range(KC):
                    if fs_hook is not None:
                        fs_hook(ck)
                    et = exp_unit(sims.pop(ck), ck, h)
                    pv_unit(fout_acc, et, ck)
                    if ck + 3 < KC:
                        sims[ck + 3] = sim_unit(psB, ck + 3, h)

            def pre_tail(h):
                # 1/Z and fout scaling; no PSUM needed beyond fout_acc.
                # recips first so the two gpsimd broadcasts run
                # back-to-back; then the two scale-muls.
                fout_acc = fout_accs[h]
                invzbs = []
                for s in range(2):
                    cs = slice(s * 512, (s + 1) * 512)
                    invz = sbM.tile([1, 512], F32, tag="zrow", bufs=2,
                                    name=f"invz_{h}_{s}")
                    nc.vector.reciprocal_approx_fast(invz[:], fout_acc[0:1, cs])
                    invzb = sbM.tile([128, 512], F32, tag="izb", bufs=2,
                                     name=f"invzb_{h}_{s}")
                    nc.gpsimd.partition_broadcast(invzb[:], invz[:])
                    invzbs.append(invzb)
                for s in range(2):
                    cs = slice(s * 512, (s + 1) * 512)
                    nc.vector.tensor_mul(
                        scaled[:, h * 1024 + s * 512:h * 1024 + (s + 1) * 512],
                        fout_acc[:, cs], invzbs[s][0:65, :])

            with tc.tile_pool(name="psA0", bufs=1, space="PSUM") as psA0:
                fout_accs[0] = psA0.tile([65, 1024], F32, name="fout0")
                with tc.tile_pool(name="psFS", bufs=1, space="PSUM") as psFS:

                    def fs_mms(ck):
                        p = psFS.tile([128, 66], F32, tag="fs", bufs=2,
                                      name=f"p_fs_{ck}")
                        nc.tensor.matmul(p[:],
                                         h1s_aug[:, ck * 128:(ck + 1) * 128],
                                         t_ws2a, start=True, stop=True)
                        nc.vector.tensor_copy(fselfT[:, ck * 65:(ck + 1) * 65],
                                              p[:, 0:65])

                    def h1s_mms(blk):
                        t_xs = xs_tiles[blk]
                        p = psFS.tile([64, 512], F32, tag="fs", bufs=2,
                                      name=f"p_h1s_{blk}")
                        for a in range(2):
                            nc.tensor.matmul(p[:],
                                             t_ws1t[:, a * 64:(a + 1) * 64],
                                             t_xs[:, a * 512:(a + 1) * 512],
                                             start=(a == 0), stop=(a == 1))
                        nc.vector.tensor_copy(
                            h1s_aug[0:64, blk * 512:blk * 512 + 512], p[:])

                    def fx_mms(blk):
                        pf = psFS.tile([128, 512], F32, tag="fs", bufs=2,
                                       name=f"p_fxL_{blk}")
                        nc.tensor.matmul(pf[:], t_bx2r, ones_r,
                                         start=True, stop=False)
                        for a in range(2):
                            nc.tensor.matmul(
                                pf[:], t_wxe[:, a * 128:(a + 1) * 128],
                                t_xl[:, blk * 1024 + a * 512:
                                     blk * 1024 + a * 512 + 512],
                                start=False, stop=(a == 1))
                        nc.vector.tensor_copy(
                            fx2[:, blk * 512:blk * 512 + 512], pf[:])

                    # h1s blocks / fself chunks paced to xs DMA arrival;
                    # fs chunk ck must complete before pv(ck).
                    H1S_AT = {5: 2, 6: 3, 9: 4, 10: 5, 12: 6, 13: 7}
                    FS_AT = {2: (4, 5), 3: (6, 7), 7: (8, 9), 8: (10, 11),
                             9: (12, 13), 10: (14, 15), 11: (16, 17),
                             12: (18, 19), 13: (20, 21), 14: (22, 23),
                             15: (24, 25), 16: (26, 27), 17: (28, 29),
                             18: (30, 31)}

                    def fs_hook(ck):
                        if ck == -1:
                            h1s_mms(0)
                            h1s_mms(1)
                            for c in range(4):
                                fs_mms(c)
                            return
                        if ck == 3:
                            fx_mms(2)
                        elif ck == 4:
                            fx_mms(3)
                        if ck in H1S_AT:
                            h1s_mms(H1S_AT[ck])
                        for c in FS_AT.get(ck, ()):
                            fs_mms(c)

                    with tc.tile_pool(name="psB0", bufs=1,
                                      space="PSUM") as psB0:
                        half_loop(0, psB0, fs_hook, preamble=True)

                with tc.tile_pool(name="psA1", bufs=1, space="PSUM") as psA1:
                    fout_accs[1] = psA1.tile([65, 1024], F32, name="fout1")
                    with tc.tile_pool(name="psB1", bufs=1,
                                      space="PSUM") as psB1:
                        pre_tail(0)
                        half_loop(1, psB1, None)
                    pre_tail(1)

                    if debug:
                        nc.sync.dma_start(d_fout[:, 0:1024], fout_accs[0][:])
                        nc.sync.dma_start(d_fout[:, 1024:2048], fout_accs[1][:])

                    # ====== final tail: up-projection + residual ======
                    dma_engines = [nc.sync, nc.scalar]
                    with tc.tile_pool(name="psC", bufs=1, space="PSUM") as psC:
                        for q in range(4):
                            cs = slice(q * 512, (q + 1) * 512)
                            for a in range(2):
                                p = psC.tile([128, 512], F32, tag="up", bufs=4,
                                             name=f"p_up_{q}_{a}")
                                nc.tensor.matmul(
                                    p[:], t_wupt[:, a * 128:(a + 1) * 128],
                                    scaled[:, cs], start=True, stop=True)
                                out_s = sbM.tile([128, 512], F32, tag="tail",
                                                 bufs=8, name=f"out_s_{q}_{a}")
                                xlv = t_xl[:, q * 1024 + a * 512:
                                           q * 1024 + a * 512 + 512]
                                nc.vector.tensor_add(out_s[:], p[:], xlv)
                                dma_engines[(2 * q + a) % 2].dma_start(
                                    out[:, a * 2048 + q * 512:
                                        a * 2048 + (q + 1) * 512], out_s[:])

            sbP1_cm.__exit__(None, None, None)
            if debug:
                nc.sync.dma_start(d_fy2[:], fy2[:])
                nc.sync.dma_start(d_fx2[:], fx2[:])
                nc.sync.dma_start(d_h1s[:], h1s_aug[:])
                nc.sync.dma_start(d_scaled[:], scaled[:])

    nc.compile()
    return nc


def _key_perm():
    """Key order: all even-w pixels (h-major), then all odd-w pixels."""
    k = np.arange(2048)
    even = (k // 32) * 64 + (k % 32) * 2
    odd = even + 1
    return np.concatenate([even, odd])


def _prep_maps(x, y, W_self1, b_self1, W_self2, b_self2, W_x1, b_x1, W_x2,
               b_x2, W_y1, b_y1, W_y2, b_y2, W_up, b_up):
    f64 = np.float64
    f16 = np.float16

    def fold(W2, b1, b2):
        return (W2.astype(f64) @ b1.astype(f64) + b2.astype(f64)).astype(np.float32)

    ws2a = np.zeros((65, 66), np.float32)
    ws2a[64, 0] = 1.0
    ws2a[0:64, 1:65] = W_self2.T
    ws2a[64, 1:65] = fold(W_self2, b_self1, b_self2)

    ws1t = np.ascontiguousarray(
        W_self1.T.reshape(2, 128, 64).transpose(1, 0, 2).reshape(128, 128))
    # fused single-layer fx / fy weights (f64 fold), K-chunked and
    # column-duplicated: [128, nchunk*128]
    wxe = (W_x2.astype(f64) @ W_x1.astype(f64)).T      # [256, 64]
    wxe = np.concatenate(
        [np.tile(wxe.reshape(2, 128, 64)[a], (1, 2)) for a in range(2)], 1)
    wye = (W_y2.astype(f64) @ W_y1.astype(f64)).T      # [512, 64]
    wye = np.concatenate(
        [np.tile(wye.reshape(4, 128, 64)[a], (1, 2)) for a in range(4)], 1)
    wupt = np.ascontiguousarray(
        np.concatenate([b_up.reshape(1, 256), W_up.T], axis=0))
    wp = np.zeros((128, _WEND), f16)
    wp[:, _WS1:_WS1 + 128] = ws1t
    wp[0:65, _WS2A:_WS2A + 66] = ws2a
    wp[:, _WXE:_WXE + 256] = wxe
    wp[:, _WYE:_WYE + 512] = wye
    wp[0:65, _WUP:_WUP + 256] = wupt
    wp[0, _WB:_WB + 128] = np.tile(fold(W_x2, b_x1, b_x2), 2)
    wp[0, _WB + 128:_WB + 256] = np.tile(fold(W_y2, b_y1, b_y2), 2)
    wp[0, _WB + 256:_WB + 768] = 1.0

    _ONES = np.ones((1, 4096), f16)
    perm = _key_perm()
    maps = []
    for b in range(B):
        xf = x[b].reshape(CX, N)                                # [256, 4096]
        xs_h = np.ascontiguousarray(
            xf[:, perm].reshape(2, 128, 8, 512).transpose(1, 2, 0, 3)
            .reshape(128, 8192)).astype(f16)
        yf = y[b].reshape(CY, NYC)
        yb_h = np.ascontiguousarray(
            yf.reshape(4, 128, 2, 512).transpose(1, 2, 0, 3).reshape(128, 4096)
        ).astype(f16)
        for half in range(2):
            xh = xf[:, half * NH:(half + 1) * NH]               # [256, 2048]
            xl_h = np.ascontiguousarray(
                xh.reshape(2, 128, 4, 512).transpose(1, 2, 0, 3).reshape(128, 4096)
            ).astype(f16)
            maps.append({
                "xs": xs_h, "xl": xl_h, "yb": yb_h,
                "wpack": wp, "ones": _ONES,
            })
    return maps


def _run(inputs, trace=False, trace_kwargs=None, debug=False):
    key = ("nc", debug)
    if key not in _CACHE:
        _CACHE[key] = _build(debug=debug)
    nc = _CACHE[key]
    maps = _prep_maps(**inputs)
    res = run_bass_kernel_spmd(nc, maps, list(range(8)), trace=trace,
                               **(trace_kwargs or {}))
    outs = np.empty((B, CX, H, W), np.float32)
    for b in range(B):
        for half in range(2):
            o = res.results[2 * b + half]["out"]                # [128, 4096]
            oh = o.reshape(128, 2, NH).transpose(1, 0, 2).reshape(CX, NH)
            outs[b, :, :, :].reshape(CX, N)[:, half * NH:(half + 1) * NH] = oh
    return outs, res


def kernel(**inputs):
    outs, _ = _run(inputs, trace=False)
    return outs


# revision 14
# speedup vs baseline: 1.3815x; 1.0083x over previous
"""Trainium2 Bass kernel for nn_BCA_17274358465235.

Module: out = x + conv1x1_up( softmax(fx @ fy_up^T) @ fself ) with
fx/fself = 2-layer 1x1-conv projections of x, fy = projection of
bilinearly-upsampled y.  B=4, CX=256, CY=512, CM=64, H=W=64 (N=4096
tokens), HY=WY=32.

Sharding: 8 cores = batch(4) x query-row-half(2).  Each core holds all
4096 keys (fy/fself replicated per batch) and 2048 query rows.  No
collectives.

Per-core algorithm (layouts chosen so no transposes are needed):
  fx / fy two-layer projections are folded on the host into single
  1x1 convs (associativity of 1x1 convs; folded in f64).  fself keeps
  the two-layer form because its second layer, emitted transposed,
  provides the [key, ch] layout plus a free all-ones Z column.
  All projection/sim matmuls run in fp16 (PE full rate + FWL); PV and
  exp outputs in bf16 (range), accumulation fp32 in PSUM.
  sim^T[key, row] = fy_f[:, keys]^T @ fx[:, rows]   (two key-chunks
      packed into PE row-groups 0-1 / 2-3 via duplicated operands)
  exp on ACT (no max-subtraction: |sim| < 70, fp32-safe)
  fout^T[c, row] += fself^T_chunk^T @ exp_chunk   (PSUM accumulation,
      ones-column in fself^T produces the softmax denominator Z free)
  out = x + W_up @ (fout^T * (1/Z)) + b_up   (b_up via ones-row in W_up)
The y-upsample runs after the channel projection (linear ops commute):
bilinear 2x with half-pixel centers == 0.25/0.75 stencil with edge
clamping, on DVE.  Keys are stored in even-w-then-odd-w order (host
permutes xs to match) so the W-pass writes are dense and 2x-capable.
Biases ride as K=1 matmuls against a ones row packed with the weights.

Scheduling: one prioritized DMA queue (fy weights + y first; the exp
marathon starts once fy band 0 and fx blocks 0-1 exist); everything
the first exp depends on is split across ACT and DVE, all later prep
(fy band 1, fx blocks 2-3, the whole fself stream) is paced through
the half-loop-0 hook to land just before its consumer, keeping ACT
100%-busy on exp.  Dummy matmuls keep the PE HAM clock warm.
"""
import sys

for _p in ("/opt/pypackages", "/opt/trn_rl_repo"):
    if _p not in sys.path:
        sys.path.insert(0, _p)

import numpy as np

import concourse.bacc as bacc
import concourse.mybir as mybir
import concourse.tile as tile
from concourse.bass_utils import run_bass_kernel_spmd

F32 = mybir.dt.float32
F16 = mybir.dt.float16
BF16 = mybir.dt.bfloat16
EXP = mybir.ActivationFunctionType.Exp
COPY = mybir.ActivationFunctionType.Copy
MUL = mybir.AluOpType.mult
ADD_ = mybir.AluOpType.add

B, CX, CY, CM = 4, 256, 512, 64
H = W = 64
HY = WY = 32
N = H * W              # 4096 tokens
NH = N // 2            # 2048 query rows per core
NYC = HY * WY          # 1024 coarse tokens
KC = N // 128          # 32 key chunks

# wpack (f16) column offsets, ordered by DMA need time.
# row 0 of _WB holds [bx2 | by2 | ones(512)].
_WYE, _WB, _WXE, _WUP, _WS1, _WS2A, _WEND = 0, 512, 1280, 1536, 1792, 1920, 1986
_WA_SPLIT = _WUP        # wpackA = [:, 0:_WA_SPLIT], wpackB = rest

_CACHE = {}


def _build(debug=False):
    nc = bacc.Bacc("TRN2", target_bir_lowering=False, debug=False,
                   enable_asserts=False)

    # ---- DRAM I/O (per-core layouts pre-arranged on host, all f16) ----
    xs = nc.dram_tensor("xs", [128, 8192], F16, kind="ExternalInput").ap()
    xl = nc.dram_tensor("xl", [128, 4096], F16, kind="ExternalInput").ap()
    yb = nc.dram_tensor("yb", [128, 4096], F16, kind="ExternalInput").ap()
    wpack = nc.dram_tensor("wpack", [128, _WEND], F16, kind="ExternalInput").ap()
    ones = nc.dram_tensor("ones", [1, 4096], F16, kind="ExternalInput").ap()
    out = nc.dram_tensor("out", [128, 4096], F32, kind="ExternalOutput").ap()
    if debug:
        d_fy2 = nc.dram_tensor("d_fy2", [128, 4096], F16, kind="ExternalOutput").ap()
        d_fx2 = nc.dram_tensor("d_fx2", [128, 2048], F16, kind="ExternalOutput").ap()
        d_h1s = nc.dram_tensor("d_h1s", [65, 4096], F16, kind="ExternalOutput").ap()
        d_sim0 = nc.dram_tensor("d_sim0", [128, 1024], F32, kind="ExternalOutput").ap()
        d_fout = nc.dram_tensor("d_fout", [65, 2048], F32, kind="ExternalOutput").ap()
        d_scaled = nc.dram_tensor("d_scaled", [65, 2048], F16, kind="ExternalOutput").ap()

    with tile.TileContext(nc) as tc:
        with tc.tile_pool(name="sbW", bufs=1) as sbW, \
             tc.tile_pool(name="sbM", bufs=1) as sbM:
            # ---- long-lived SBUF ----
            t_xl = sbM.tile([128, 4096], F16)      # fx input + residual
            fy2 = sbM.tile([128, 4096], F16)       # upsampled fy (even|odd)
            fx2 = sbM.tile([128, 2048], F16)       # fx, duplicated
            fselfT = sbM.tile([128, 65 * KC], BF16)
            h1s_aug = sbM.tile([65, 4096], F16)    # W_self1 @ x with ones row
            scaled = sbM.tile([65, 2048], F16)     # [Z/Z; fout/Z] per row

            t_wpack = sbW.tile([128, _WEND], F16)
            t_wye = t_wpack[:, _WYE:_WYE + 512]
            t_bx2r = t_wpack[0:1, _WB:_WB + 128]
            t_by2r = t_wpack[0:1, _WB + 128:_WB + 256]
            ones_r = t_wpack[0:1, _WB + 256:_WB + 768]
            t_wxe = t_wpack[:, _WXE:_WXE + 256]
            t_wupt = t_wpack[0:65, _WUP:_WUP + 256]
            t_ws1t = t_wpack[:, _WS1:_WS1 + 128]
            t_ws2a = t_wpack[0:65, _WS2A:_WS2A + 66]

            sbP1_cm = tc.tile_pool(name="sbP1", bufs=1)
            sbP1 = sbP1_cm.__enter__()
            t_yb = sbP1.tile([128, 4096], F16)
            xs_all = sbP1.tile([128, 8192], F16)
            xs_tiles = [xs_all[:, b * 1024:(b + 1) * 1024] for b in range(8)]
            fyc75 = sbP1.tile([128, 1024], F16)
            fyc25 = sbP1.tile([128, 1024], F16)
            fyH = sbP1.tile([128, 2048], F16)
            u2 = sbP1.tile([128, 2048], F16)

            # ---- input DMAs: one queue, ordered by need time ----
            nc.sync.dma_start(t_wpack[:, 0:_WA_SPLIT], wpack[:, 0:_WA_SPLIT])
            nc.sync.dma_start(t_yb[:, 0:1024], yb[:, 0:1024])
            nc.sync.dma_start(t_yb[:, 1024:2048], yb[:, 1024:2048])
            nc.sync.dma_start(t_xl[:, 0:2048], xl[:, 0:2048])
            nc.sync.dma_start(t_yb[:, 2048:4096], yb[:, 2048:4096])
            nc.sync.dma_start(xs_all[:, 0:2048], xs[:, 0:2048])
            nc.sync.dma_start(t_wpack[:, _WA_SPLIT:_WEND],
                              wpack[:, _WA_SPLIT:_WEND])
            nc.sync.dma_start(xs_all[:, 2048:4096], xs[:, 2048:4096])
            nc.sync.dma_start(t_xl[:, 2048:4096], xl[:, 2048:4096])
            nc.sync.dma_start(xs_all[:, 4096:6144], xs[:, 4096:6144])
            nc.sync.dma_start(xs_all[:, 6144:8192], xs[:, 6144:8192])
            nc.gpsimd.dma_start(h1s_aug[64:65, :], ones[:, 0:4096])

            # views for the upsample chain
            t1v = fyc75[:].rearrange("p (h w) -> p h w", h=32)
            t2v = fyc25[:].rearrange("p (h w) -> p h w", h=32)
            fe = fyH[:].rearrange("p (h two w) -> p h two w", h=32, two=2)
            u2v = u2[:].rearrange("p (h w) -> p h w", h=64)
            fyHv = fyH[:].rearrange("p (h w) -> p h w", h=64)
            f2e = fy2[:, 0:2048].rearrange("p (h w) -> p h w", h=64)
            f2o = fy2[:, 2048:4096].rearrange("p (h w) -> p h w", h=64)

            def w_pass(which, hs):
                if which == 0:
                    nc.vector.scalar_tensor_tensor(
                        f2e[:, hs, 1:32], fyHv[:, hs, 1:32], 0.75,
                        u2v[:, hs, 0:31], MUL, ADD_)
                    nc.vector.tensor_copy(f2e[:, hs, 0], fyHv[:, hs, 0])
                else:
                    nc.vector.scalar_tensor_tensor(
                        f2o[:, hs, 0:31], fyHv[:, hs, 0:31], 0.75,
                        u2v[:, hs, 1:32], MUL, ADD_)
                    nc.vector.tensor_copy(f2o[:, hs, 31], fyHv[:, hs, 31])

            b0, b1 = slice(0, 31), slice(31, 64)

            # ---- phase 1: band-0 fy chain + fx blocks 0-1 ----
            with tc.tile_pool(name="psP1", bufs=1, space="PSUM") as psP1:
                # warm the ACT exp table + the PE HAM clock early
                t_dum = sbP1.tile([1, 32], F32)
                t_warm = sbP1.tile([64, 512], F16)
                nc.vector.memset(t_warm[:], 0.25)
                nc.vector.memset(t_dum[:], 0.0)
                t_dum2 = sbP1.tile([1, 32], F32)
                nc.scalar.activation(t_dum2[:], t_dum[:], EXP)
                p_warm = psP1.tile([128, 512], F32, tag="warm", name="p_warm")
                for _w in range(6):
                    nc.tensor.matmul(p_warm[:], t_warm[:, 0:128], t_warm[:],
                                     start=True, stop=True)

                # fyc block 0 (coarse rows 0:16); a-chunks pipeline with
                # the two yb half-0 DMA pieces
                p_fyc0 = psP1.tile([128, 512], F32, tag="blk", bufs=4,
                                   name="p_fyc_0")
                nc.tensor.matmul(p_fyc0[:], t_by2r, ones_r,
                                 start=True, stop=False)
                for a in range(4):
                    nc.tensor.matmul(
                        p_fyc0[:], t_wye[:, a * 128:(a + 1) * 128],
                        t_yb[:, a * 512:a * 512 + 512],
                        start=False, stop=(a == 3))
                # fx blocks 0-1 (xl half 0)
                fx_ps = []
                for fblk in range(2):
                    pf = psP1.tile([128, 512], F32, tag="blk", bufs=4,
                                   name=f"p_fx_{fblk}")
                    nc.tensor.matmul(pf[:], t_bx2r, ones_r,
                                     start=True, stop=False)
                    for a in range(2):
                        nc.tensor.matmul(
                            pf[:], t_wxe[:, a * 128:(a + 1) * 128],
                            t_xl[:, fblk * 1024 + a * 512:
                                 fblk * 1024 + a * 512 + 512],
                            start=False, stop=(a == 1))
                    fx_ps.append(pf)

                # band-0 chain split across ACT (75-copy, u2, fx0) and
                # DVE (25-copy, H pass, W pass, fx1); all of this
                # precedes exp0 on both FIFOs.
                nc.scalar.activation(fyc75[:, 0:512], p_fyc0[:], COPY,
                                     scale=0.75)
                nc.vector.tensor_scalar_mul(fyc25[:, 0:512], p_fyc0[:], 0.25)
                nc.vector.tensor_add(fe[:, 0, 0, :], t1v[:, 0, :], t2v[:, 0, :])
                nc.vector.tensor_add(fe[:, 1:16, 0, :], t1v[:, 1:16, :], t2v[:, 0:15, :])
                nc.vector.tensor_add(fe[:, 0:15, 1, :], t1v[:, 0:15, :], t2v[:, 1:16, :])
                nc.scalar.activation(u2[:, 0:31 * 32], fyH[:, 0:31 * 32],
                                     COPY, scale=0.25)
                nc.scalar.activation(fx2[:, 0:512], fx_ps[0][:], COPY)
                w_pass(0, b0)
                nc.vector.tensor_copy(fx2[:, 512:1024], fx_ps[1][:])

            # ================= phase 2: attention (two half-loops) ====
            fout_accs = {}

            def sim_unit(pool, ck, h):
                ps = pool.tile([128, 1024], F32, tag="sim", bufs=2,
                               name=f"sim_{ck}_{h}")
                nc.tensor.matmul(
                    ps[:, 0:512], fy2[0:64, ck * 128:(ck + 1) * 128],
                    fx2[0:64, h * 1024:h * 1024 + 512],
                    start=True, stop=True)
                nc.tensor.matmul(
                    ps[:, 512:1024], fy2[64:128, ck * 128:(ck + 1) * 128],
                    fx2[64:128, h * 1024 + 512:h * 1024 + 1024],
                    start=True, stop=True)
                return ps

            def exp_unit(st, ck, h):
                et = sbM.tile([128, 1024], BF16, tag="et",
                              bufs=3 if debug else 6, name=f"et_{ck}_{h}")
                if debug and ck == 0 and h == 0:
                    d0 = sbM.tile([128, 1024], F32)
                    nc.vector.tensor_copy(d0[:], st[:])
                    nc.sync.dma_start(d_sim0[:], d0[:])
                nc.scalar.activation(et[:], st[:], EXP)
                return et

            def pv_unit(fout_acc, et, ck):
                w = fselfT[:, ck * 65:(ck + 1) * 65]
                nc.tensor.matmul(fout_acc[:, 0:512], w, et[:, 0:512],
                                 start=(ck == 0), stop=(ck == KC - 1))
                nc.tensor.matmul(fout_acc[:, 512:1024], w, et[:, 512:1024],
                                 start=(ck == 0), stop=(ck == KC - 1))

            def half_loop(h, psB, fs_hook, preamble=False):
                fout_acc = fout_accs[h]
                sims = {}
                sims[0] = sim_unit(psB, 0, h)
                sims[1] = sim_unit(psB, 1, h)
                sims[2] = sim_unit(psB, 2, h)
                if preamble:
                    fs_hook(-1)
                for ck in range(KC):
                    if fs_hook is not None:
                        fs_hook(ck)
                    et = exp_unit(sims.pop(ck), ck, h)
                    pv_unit(fout_acc, et, ck)
                    if ck + 3 < KC:
                        sims[ck + 3] = sim_unit(psB, ck + 3, h)

            def pre_tail(h):
                fout_acc = fout_accs[h]
                invzbs = []
                for s in range(2):
                    cs = slice(s * 512, (s + 1) * 512)
                    invz = sbM.tile([1, 512], F32, tag="zrow", bufs=2,
                                    name=f"invz_{h}_{s}")
                    nc.vector.reciprocal_approx_fast(invz[:], fout_acc[0:1, cs])
                    invzb = sbM.tile([128, 512], F32, tag="izb", bufs=2,
                                     name=f"invzb_{h}_{s}")
                    nc.gpsimd.partition_broadcast(invzb[:], invz[:])
                    invzbs.append(invzb)
                for s in range(2):
                    cs = slice(s * 512, (s + 1) * 512)
                    nc.vector.tensor_mul(
                        scaled[:, h * 1024 + s * 512:h * 1024 + (s + 1) * 512],
                        fout_acc[:, cs], invzbs[s][0:65, :])

            with tc.tile_pool(name="psA0", bufs=1, space="PSUM") as psA0:
                fout_accs[0] = psA0.tile([65, 1024], F32, name="fout0")
                with tc.tile_pool(name="psFS", bufs=1, space="PSUM") as psFS:

                    def fs_mms(ck):
                        p = psFS.tile([128, 66], F32, tag="fs", bufs=2,
                                      name=f"p_fs_{ck}")
                        nc.tensor.matmul(p[:],
                                         h1s_aug[:, ck * 128:(ck + 1) * 128],
                                         t_ws2a, start=True, stop=True)
                        nc.vector.tensor_copy(fselfT[:, ck * 65:(ck + 1) * 65],
                                              p[:, 0:65])

                    def h1s_mms(blk):
                        t_xs = xs_tiles[blk]
                        p = psFS.tile([64, 512], F32, tag="fs", bufs=2,
                                      name=f"p_h1s_{blk}")
                        for a in range(2):
                            nc.tensor.matmul(p[:],
                                             t_ws1t[:, a * 64:(a + 1) * 64],
                                             t_xs[:, a * 512:(a + 1) * 512],
                                             start=(a == 0), stop=(a == 1))
                        nc.vector.tensor_copy(
                            h1s_aug[0:64, blk * 512:blk * 512 + 512], p[:])

                    def fx_mms(blk):
                        pf = psFS.tile([128, 512], F32, tag="fs", bufs=2,
                                       name=f"p_fxL_{blk}")
                        nc.tensor.matmul(pf[:], t_bx2r, ones_r,
                                         start=True, stop=False)
                        for a in range(2):
                            nc.tensor.matmul(
                                pf[:], t_wxe[:, a * 128:(a + 1) * 128],
                                t_xl[:, blk * 1024 + a * 512:
                                     blk * 1024 + a * 512 + 512],
                                start=False, stop=(a == 1))
                        nc.vector.tensor_copy(
                            fx2[:, blk * 512:blk * 512 + 512], pf[:])

                    def fy_band1():
                        # fyc block 1 + H band 1 + u2 band 1 + W passes
                        p = psFS.tile([128, 512], F32, tag="fs", bufs=2,
                                      name="p_fyc_1")
                        nc.tensor.matmul(p[:], t_by2r, ones_r,
                                         start=True, stop=False)
                        for a in range(4):
                            nc.tensor.matmul(
                                p[:], t_wye[:, a * 128:(a + 1) * 128],
                                t_yb[:, 2048 + a * 512:2048 + a * 512 + 512],
                                start=False, stop=(a == 3))
                        nc.vector.tensor_scalar_mul(fyc75[:, 512:1024],
                                                    p[:], 0.75)
                        nc.vector.tensor_scalar_mul(fyc25[:, 512:1024],
                                                    p[:], 0.25)
                        nc.vector.tensor_add(fe[:, 16:32, 0, :],
                                             t1v[:, 16:32, :], t2v[:, 15:31, :])
                        nc.vector.tensor_add(fe[:, 15:31, 1, :],
                                             t1v[:, 15:31, :], t2v[:, 16:32, :])
                        nc.vector.tensor_add(fe[:, 31, 1, :],
                                             t1v[:, 31, :], t2v[:, 31, :])
                        nc.vector.tensor_scalar_mul(u2[:, 31 * 32:2048],
                                                    fyH[:, 31 * 32:2048], 0.25)
                        w_pass(0, b1)
                        w_pass(1, b0)
                        w_pass(1, b1)

                    # h1s blocks / fx blocks / fself chunks paced to the
                    # DMA arrival order; fs chunk ck completes well
                    # before pv(ck) (et bufs give ~6 chunks of slack).
                    H1S_AT = {4: 2, 5: 3, 8: 4, 9: 5, 10: 6, 11: 7}
                    FX_AT = {6: 2, 7: 3}
                    FS_AT = {2: (4, 5), 3: (6, 7), 5: (8, 9), 6: (10, 11),
                             7: (12, 13), 8: (14, 15), 9: (16, 17),
                             10: (18, 19), 11: (20, 21), 12: (22, 23),
                             13: (24, 25), 14: (26, 27), 15: (28, 29),
                             16: (30, 31)}

                    def fs_hook(ck):
                        if ck == -1:
                            fy_band1()
                            h1s_mms(0)
                            h1s_mms(1)
                            for c in range(4):
                                fs_mms(c)
                            return
                        if ck in FX_AT:
                            fx_mms(FX_AT[ck])
                        if ck in H1S_AT:
                            h1s_mms(H1S_AT[ck])
                        for c in FS_AT.get(ck, ()):
                            fs_mms(c)

                    with tc.tile_pool(name="psB0", bufs=1,
                                      space="PSUM") as psB0:
                        half_loop(0, psB0, fs_hook, preamble=True)

                with tc.tile_pool(name="psA1", bufs=1, space="PSUM") as psA1:
                    fout_accs[1] = psA1.tile([65, 1024], F32, name="fout1")
                    with tc.tile_pool(name="psB1", bufs=1,
                                      space="PSUM") as psB1:
                        pre_tail(0)
                        half_loop(1, psB1, None)
                    pre_tail(1)

                    if debug:
                        nc.sync.dma_start(d_fout[:, 0:1024], fout_accs[0][:])
                        nc.sync.dma_start(d_fout[:, 1024:2048], fout_accs[1][:])

                    # ====== final tail: up-projection + residual ======
                    dma_engines = [nc.sync, nc.scalar]
                    with tc.tile_pool(name="psC", bufs=1, space="PSUM") as psC:
                        for q in range(4):
                            cs = slice(q * 512, (q + 1) * 512)
                            for a in range(2):
                                p = psC.tile([128, 512], F32, tag="up", bufs=4,
                                             name=f"p_up_{q}_{a}")
                                nc.tensor.matmul(
                                    p[:], t_wupt[:, a * 128:(a + 1) * 128],
                                    scaled[:, cs], start=True, stop=True)
                                out_s = sbM.tile([128, 512], F32, tag="tail",
                                                 bufs=8, name=f"out_s_{q}_{a}")
                                xlv = t_xl[:, q * 1024 + a * 512:
                                           q * 1024 + a * 512 + 512]
                                nc.vector.tensor_add(out_s[:], p[:], xlv)
                                dma_engines[(2 * q + a) % 2].dma_start(
                                    out[:, a * 2048 + q * 512:
                                        a * 2048 + (q + 1) * 512], out_s[:])

            sbP1_cm.__exit__(None, None, None)
            if debug:
                nc.sync.dma_start(d_fy2[:], fy2[:])
                nc.sync.dma_start(d_fx2[:], fx2[:])
                nc.sync.dma_start(d_h1s[:], h1s_aug[:])
                nc.sync.dma_start(d_scaled[:], scaled[:])

    nc.compile()
    return nc


def _key_perm():
    """Key order: all even-w pixels (h-major), then all odd-w pixels."""
    k = np.arange(2048)
    even = (k // 32) * 64 + (k % 32) * 2
    odd = even + 1
    return np.concatenate([even, odd])


def _prep_maps(x, y, W_self1, b_self1, W_self2, b_self2, W_x1, b_x1, W_x2,
               b_x2, W_y1, b_y1, W_y2, b_y2, W_up, b_up):
    f64 = np.float64
    f16 = np.float16

    def fold(W2, b1, b2):
        return (W2.astype(f64) @ b1.astype(f64) + b2.astype(f64)).astype(np.float32)

    ws2a = np.zeros((65, 66), np.float32)
    ws2a[64, 0] = 1.0
    ws2a[0:64, 1:65] = W_self2.T
    ws2a[64, 1:65] = fold(W_self2, b_self1, b_self2)

    ws1t = np.ascontiguousarray(
        W_self1.T.reshape(2, 128, 64).transpose(1, 0, 2).reshape(128, 128))
    wxe = (W_x2.astype(f64) @ W_x1.astype(f64)).T      # [256, 64]
    wxe = np.concatenate(
        [np.tile(wxe.reshape(2, 128, 64)[a], (1, 2)) for a in range(2)], 1)
    wye = (W_y2.astype(f64) @ W_y1.astype(f64)).T      # [512, 64]
    wye = np.concatenate(
        [np.tile(wye.reshape(4, 128, 64)[a], (1, 2)) for a in range(4)], 1)
    wupt = np.ascontiguousarray(
        np.concatenate([b_up.reshape(1, 256), W_up.T], axis=0))
    wp = np.zeros((128, _WEND), f16)
    wp[:, _WYE:_WYE + 512] = wye
    wp[0, _WB:_WB + 128] = np.tile(fold(W_x2, b_x1, b_x2), 2)
    wp[0, _WB + 128:_WB + 256] = np.tile(fold(W_y2, b_y1, b_y2), 2)
    wp[0, _WB + 256:_WB + 768] = 1.0
    wp[:, _WXE:_WXE + 256] = wxe
    wp[0:65, _WUP:_WUP + 256] = wupt
    wp[:, _WS1:_WS1 + 128] = ws1t
    wp[0:65, _WS2A:_WS2A + 66] = ws2a

    _ONES = np.ones((1, 4096), f16)
    perm = _key_perm()
    maps = []
    for b in range(B):
        xf = x[b].reshape(CX, N)                                # [256, 4096]
        xs_h = np.ascontiguousarray(
            xf[:, perm].reshape(2, 128, 8, 512).transpose(1, 2, 0, 3)
            .reshape(128, 8192)).astype(f16)
        yf = y[b].reshape(CY, NYC)
        yb_h = np.ascontiguousarray(
            yf.reshape(4, 128, 2, 512).transpose(1, 2, 0, 3).reshape(128, 4096)
        ).astype(f16)
        for half in range(2):
            xh = xf[:, half * NH:(half + 1) * NH]               # [256, 2048]
            xl_h = np.ascontiguousarray(
                xh.reshape(2, 128, 4, 512).transpose(1, 2, 0, 3).reshape(128, 4096)
            ).astype(f16)
            maps.append({
                "xs": xs_h, "xl": xl_h, "yb": yb_h,
                "wpack": wp, "ones": _ONES,
            })
    return maps


def _run(inputs, trace=False, trace_kwargs=None, debug=False):
    key = ("nc", debug)
    if key not in _CACHE:
        _CACHE[key] = _build(debug=debug)
    nc = _CACHE[key]
    maps = _prep_maps(**inputs)
    res = run_bass_kernel_spmd(nc, maps, list(range(8)), trace=trace,
                               **(trace_kwargs or {}))
    outs = np.empty((B, CX, H, W), np.float32)
    for b in range(B):
        for half in range(2):
            o = res.results[2 * b + half]["out"]                # [128, 4096]
            oh = o.reshape(128, 2, NH).transpose(1, 0, 2).reshape(CX, NH)
            outs[b, :, :, :].reshape(CX, N)[:, half * NH:(half + 1) * NH] = oh
    return outs, res


def kernel(**inputs):
    outs, _ = _run(inputs, trace=False)
    return outs


# revision 18
# speedup vs baseline: 1.4474x; 1.0477x over previous
"""Trainium2 Bass kernel for nn_BCA_17274358465235.

Module: out = x + conv1x1_up( softmax(fx @ fy_up^T) @ fself ) with
fx/fself = 2-layer 1x1-conv projections of x, fy = projection of
bilinearly-upsampled y.  B=4, CX=256, CY=512, CM=64, H=W=64 (N=4096
tokens), HY=WY=32.

Sharding: 8 cores = batch(4) x query-row-half(2).  Each core holds all
4096 keys (fy/fself replicated per batch) and 2048 query rows.  No
collectives.

Per-core algorithm (layouts chosen so no transposes are needed):
  fx / fy two-layer projections are folded on the host into single
  1x1 convs (associativity of 1x1 convs; folded in f64).  fself keeps
  the two-layer form because its second layer, emitted transposed,
  provides the [key, ch] layout plus a free all-ones Z column.
  All projection/sim matmuls run in fp16 (PE full rate + FWL); PV and
  exp outputs in bf16 (range), accumulation fp32 in PSUM.
  sim^T[key, row] = fy_f[:, keys]^T @ fx[:, rows]   (two key-chunks
      packed into PE row-groups 0-1 / 2-3 via duplicated operands)
  exp on ACT (no max-subtraction: |sim| < 70, fp32-safe)
  fout^T[c, row] += fself^T_chunk^T @ exp_chunk   (PSUM accumulation,
      ones-column in fself^T produces the softmax denominator Z free)
  out = x + W_up @ (fout^T * (1/Z)) + b_up   (b_up via ones-row in W_up)
The y-upsample runs after the channel projection (linear ops commute):
bilinear 2x with half-pixel centers == 0.25/0.75 stencil with edge
clamping, on DVE.  Keys are stored in even-w-then-odd-w order (host
permutes xs to match) so the W-pass writes are dense and 2x-capable.
Biases ride as K=1 matmuls against a ones row packed with the weights.

Scheduling: one prioritized DMA queue (fy weights + y first; the exp
marathon starts once fy band 0 and fx blocks 0-1 exist); everything
the first exp depends on is split across ACT and DVE, all later prep
(fy band 1, fx blocks 2-3, the whole fself stream) is paced through
the half-loop-0 hook to land just before its consumer, keeping ACT
100%-busy on exp.  Dummy matmuls keep the PE HAM clock warm.
"""
import sys

for _p in ("/opt/pypackages", "/opt/trn_rl_repo"):
    if _p not in sys.path:
        sys.path.insert(0, _p)

import numpy as np

import concourse.bacc as bacc
import concourse.mybir as mybir
import concourse.tile as tile
from concourse.bass_utils import run_bass_kernel_spmd

F32 = mybir.dt.float32
F16 = mybir.dt.float16
BF16 = mybir.dt.bfloat16
EXP = mybir.ActivationFunctionType.Exp
COPY = mybir.ActivationFunctionType.Copy
MUL = mybir.AluOpType.mult
ADD_ = mybir.AluOpType.add

B, CX, CY, CM = 4, 256, 512, 64
H = W = 64
HY = WY = 32
N = H * W              # 4096 tokens
NH = N // 2            # 2048 query rows per core
NYC = HY * WY          # 1024 coarse tokens
KC = N // 128          # 32 key chunks

# wpack (f16) column offsets, ordered by DMA need time.
# row 0 of _WB holds [bx2 | by2 | ones(512)].
_WYE, _WB, _WXE, _WUP, _WS1, _WS2A, _WEND = 0, 512, 1280, 1536, 1792, 1920, 1986
_WA_SPLIT = _WUP        # wpackA = [:, 0:_WA_SPLIT], wpackB = rest

_CACHE = {}


def _build(debug=False):
    nc = bacc.Bacc("TRN2", target_bir_lowering=False, debug=False,
                   enable_asserts=False)

    # ---- DRAM I/O (per-core layouts pre-arranged on host, all f16) ----
    xs = nc.dram_tensor("xs", [128, 8192], F16, kind="ExternalInput").ap()
    xl = nc.dram_tensor("xl", [128, 4096], F16, kind="ExternalInput").ap()
    yb = nc.dram_tensor("yb", [128, 4096], F16, kind="ExternalInput").ap()
    wpack = nc.dram_tensor("wpack", [128, _WEND], F16, kind="ExternalInput").ap()
    ones = nc.dram_tensor("ones", [1, 4096], F16, kind="ExternalInput").ap()
    out = nc.dram_tensor("out", [128, 4096], F32, kind="ExternalOutput").ap()
    if debug:
        d_fy2 = nc.dram_tensor("d_fy2", [128, 4096], F16, kind="ExternalOutput").ap()
        d_fx2 = nc.dram_tensor("d_fx2", [128, 2048], F16, kind="ExternalOutput").ap()
        d_h1s = nc.dram_tensor("d_h1s", [65, 4096], F16, kind="ExternalOutput").ap()
        d_sim0 = nc.dram_tensor("d_sim0", [128, 1024], F32, kind="ExternalOutput").ap()
        d_fout = nc.dram_tensor("d_fout", [65, 2048], F32, kind="ExternalOutput").ap()
        d_scaled = nc.dram_tensor("d_scaled", [65, 2048], F16, kind="ExternalOutput").ap()

    with tile.TileContext(nc) as tc:
        with tc.tile_pool(name="sbW", bufs=1) as sbW, \
             tc.tile_pool(name="sbM", bufs=1) as sbM:
            # ---- long-lived SBUF ----
            t_xl = sbM.tile([128, 4096], F16)      # fx input + residual
            fy2 = sbM.tile([128, 4096], F16)       # upsampled fy (even|odd)
            fx2 = sbM.tile([128, 2048], F16)       # fx, duplicated
            fselfT = sbM.tile([128, 65 * KC], BF16)
            h1s_aug = sbM.tile([65, 4096], F16)    # W_self1 @ x with ones row
            scaled = sbM.tile([65, 2048], F16)     # [Z/Z; fout/Z] per row

            t_wpack = sbW.tile([128, _WEND], F16)
            t_wye = t_wpack[:, _WYE:_WYE + 512]
            t_bx2r = t_wpack[0:1, _WB:_WB + 128]
            t_by2r = t_wpack[0:1, _WB + 128:_WB + 256]
            ones_r = t_wpack[0:1, _WB + 256:_WB + 768]
            t_wxe = t_wpack[:, _WXE:_WXE + 256]
            t_wupt = t_wpack[0:65, _WUP:_WUP + 256]
            t_ws1t = t_wpack[:, _WS1:_WS1 + 128]
            t_ws2a = t_wpack[0:65, _WS2A:_WS2A + 66]

            sbP1_cm = tc.tile_pool(name="sbP1", bufs=1)
            sbP1 = sbP1_cm.__enter__()
            t_yb = sbP1.tile([128, 4096], F16)
            xs_all = sbP1.tile([128, 8192], F16)
            xs_tiles = [xs_all[:, b * 1024:(b + 1) * 1024] for b in range(8)]
            fyc75 = sbP1.tile([128, 1024], F16)
            fyc25 = sbP1.tile([128, 1024], F16)
            fyH = sbP1.tile([128, 2048], F16)
            u2 = sbP1.tile([128, 2048], F16)

            # ---- input DMAs: one queue, ordered by need time ----
            nc.sync.dma_start(t_wpack[:, 0:_WA_SPLIT], wpack[:, 0:_WA_SPLIT])
            nc.sync.dma_start(t_yb[:, 0:1024], yb[:, 0:1024])
            nc.sync.dma_start(t_yb[:, 1024:2048], yb[:, 1024:2048])
            nc.sync.dma_start(t_xl[:, 0:2048], xl[:, 0:2048])
            nc.sync.dma_start(t_yb[:, 2048:4096], yb[:, 2048:4096])
            nc.sync.dma_start(t_wpack[:, _WA_SPLIT:_WEND],
                              wpack[:, _WA_SPLIT:_WEND])
            nc.sync.dma_start(xs_all[:, 0:2048], xs[:, 0:2048])
            nc.sync.dma_start(xs_all[:, 2048:4096], xs[:, 2048:4096])
            nc.sync.dma_start(t_xl[:, 2048:4096], xl[:, 2048:4096])
            nc.sync.dma_start(xs_all[:, 4096:6144], xs[:, 4096:6144])
            nc.sync.dma_start(xs_all[:, 6144:8192], xs[:, 6144:8192])
            nc.gpsimd.dma_start(h1s_aug[64:65, :], ones[:, 0:4096])

            # views for the upsample chain
            t1v = fyc75[:].rearrange("p (h w) -> p h w", h=32)
            t2v = fyc25[:].rearrange("p (h w) -> p h w", h=32)
            fe = fyH[:].rearrange("p (h two w) -> p h two w", h=32, two=2)
            u2v = u2[:].rearrange("p (h w) -> p h w", h=64)
            fyHv = fyH[:].rearrange("p (h w) -> p h w", h=64)
            f2e = fy2[:, 0:2048].rearrange("p (h w) -> p h w", h=64)
            f2o = fy2[:, 2048:4096].rearrange("p (h w) -> p h w", h=64)

            def w_pass(which, hs):
                if which == 0:
                    nc.vector.scalar_tensor_tensor(
                        f2e[:, hs, 1:32], fyHv[:, hs, 1:32], 0.75,
                        u2v[:, hs, 0:31], MUL, ADD_)
                    nc.vector.tensor_copy(f2e[:, hs, 0], fyHv[:, hs, 0])
                else:
                    nc.vector.scalar_tensor_tensor(
                        f2o[:, hs, 0:31], fyHv[:, hs, 0:31], 0.75,
                        u2v[:, hs, 1:32], MUL, ADD_)
                    nc.vector.tensor_copy(f2o[:, hs, 31], fyHv[:, hs, 31])

            b0, b1 = slice(0, 31), slice(31, 64)

            # ---- phase 1: band-0 fy chain + fx blocks 0-1 ----
            with tc.tile_pool(name="psP1", bufs=1, space="PSUM") as psP1:
                # warm the ACT exp table + the PE HAM clock early
                t_dum = sbP1.tile([1, 32], F32)
                t_warm = sbP1.tile([64, 512], F16)
                nc.vector.memset(t_warm[:], 0.25)
                nc.vector.memset(t_dum[:], 0.0)
                t_dum2 = sbP1.tile([1, 32], F32)
                nc.scalar.activation(t_dum2[:], t_dum[:], EXP)
                p_warm = psP1.tile([128, 512], F32, tag="warm", name="p_warm")
                for _w in range(10):
                    nc.tensor.matmul(p_warm[:], t_warm[:, 0:128], t_warm[:],
                                     start=True, stop=True)

                # fyc block 0 (coarse rows 0:16); a-chunks pipeline with
                # the two yb half-0 DMA pieces
                p_fyc0 = psP1.tile([128, 512], F32, tag="blk", bufs=4,
                                   name="p_fyc_0")
                nc.tensor.matmul(p_fyc0[:], t_by2r, ones_r,
                                 start=True, stop=False)
                for a in range(4):
                    nc.tensor.matmul(
                        p_fyc0[:], t_wye[:, a * 128:(a + 1) * 128],
                        t_yb[:, a * 512:a * 512 + 512],
                        start=False, stop=(a == 3))
                # fx blocks 0-1 (xl half 0)
                fx_ps = []
                for fblk in range(2):
                    pf = psP1.tile([128, 512], F32, tag="blk", bufs=4,
                                   name=f"p_fx_{fblk}")
                    nc.tensor.matmul(pf[:], t_bx2r, ones_r,
                                     start=True, stop=False)
                    for a in range(2):
                        nc.tensor.matmul(
                            pf[:], t_wxe[:, a * 128:(a + 1) * 128],
                            t_xl[:, fblk * 1024 + a * 512:
                                 fblk * 1024 + a * 512 + 512],
                            start=False, stop=(a == 1))
                    fx_ps.append(pf)

                # band-0 chain split across ACT (75-copy, u2, fx0) and
                # DVE (25-copy, H pass, W pass, fx1); all of this
                # precedes exp0 on both FIFOs.
                nc.scalar.activation(fyc75[:, 0:512], p_fyc0[:], COPY,
                                     scale=0.75)
                nc.vector.tensor_scalar_mul(fyc25[:, 0:512], p_fyc0[:], 0.25)
                nc.vector.tensor_add(fe[:, 0, 0, :], t1v[:, 0, :], t2v[:, 0, :])
                nc.vector.tensor_add(fe[:, 1:16, 0, :], t1v[:, 1:16, :], t2v[:, 0:15, :])
                nc.vector.tensor_add(fe[:, 0:15, 1, :], t1v[:, 0:15, :], t2v[:, 1:16, :])
                nc.scalar.activation(u2[:, 0:31 * 32], fyH[:, 0:31 * 32],
                                     COPY, scale=0.25)
                nc.scalar.activation(fx2[:, 0:512], fx_ps[0][:], COPY)
                w_pass(0, b0)
                nc.vector.tensor_copy(fx2[:, 512:1024], fx_ps[1][:])

            # ================= phase 2: attention (two half-loops) ====
            fout_accs = {}

            def sim_unit(pool, ck, h):
                ps = pool.tile([128, 1024], F32, tag="sim", bufs=2,
                               name=f"sim_{ck}_{h}")
                nc.tensor.matmul(
                    ps[:, 0:512], fy2[0:64, ck * 128:(ck + 1) * 128],
                    fx2[0:64, h * 1024:h * 1024 + 512],
                    start=True, stop=True)
                nc.tensor.matmul(
                    ps[:, 512:1024], fy2[64:128, ck * 128:(ck + 1) * 128],
                    fx2[64:128, h * 1024 + 512:h * 1024 + 1024],
                    start=True, stop=True)
                return ps

            def exp_unit(st, ck, h):
                et = sbM.tile([128, 1024], BF16, tag="et",
                              bufs=3 if debug else 12, name=f"et_{ck}_{h}")
                if debug and ck == 0 and h == 0:
                    d0 = sbM.tile([128, 1024], F32)
                    nc.vector.tensor_copy(d0[:], st[:])
                    nc.sync.dma_start(d_sim0[:], d0[:])
                nc.scalar.activation(et[:], st[:], EXP)
                return et

            def pv_unit(fout_acc, et, ck):
                w = fselfT[:, ck * 65:(ck + 1) * 65]
                nc.tensor.matmul(fout_acc[:, 0:512], w, et[:, 0:512],
                                 start=(ck == 0), stop=(ck == KC - 1))
                nc.tensor.matmul(fout_acc[:, 512:1024], w, et[:, 512:1024],
                                 start=(ck == 0), stop=(ck == KC - 1))

            # pv emission is deferred `defer` chunks behind exp so the
            # xs-DMA-paced fself stream never stalls the sim/exp FIFO;
            # leftovers drain into the next half-loop.
            pending_pvs = []

            def flush_pvs(keep):
                while len(pending_pvs) > keep:
                    acc, et, ck = pending_pvs.pop(0)
                    pv_unit(acc, et, ck)

            def half_loop(h, psB, fs_hook, defer=0):
                fout_acc = fout_accs[h]
                sims = {}
                sims[0] = sim_unit(psB, 0, h)
                sims[1] = sim_unit(psB, 1, h)
                sims[2] = sim_unit(psB, 2, h)
                if fs_hook is not None:
                    fs_hook(-1)
                for ck in range(KC):
                    if fs_hook is not None:
                        fs_hook(ck)
                    et = exp_unit(sims.pop(ck), ck, h)
                    pending_pvs.append((fout_acc, et, ck))
                    flush_pvs(defer)
                    if ck + 3 < KC:
                        sims[ck + 3] = sim_unit(psB, ck + 3, h)

            def pre_tail(h):
                fout_acc = fout_accs[h]
                invzbs = []
                for s in range(2):
                    cs = slice(s * 512, (s + 1) * 512)
                    invz = sbM.tile([1, 512], F32, tag="zrow", bufs=2,
                                    name=f"invz_{h}_{s}")
                    nc.vector.reciprocal_approx_fast(invz[:], fout_acc[0:1, cs])
                    invzb = sbM.tile([128, 512], F32, tag="izb", bufs=2,
                                     name=f"invzb_{h}_{s}")
                    nc.gpsimd.partition_broadcast(invzb[:], invz[:])
                    invzbs.append(invzb)
                for s in range(2):
                    cs = slice(s * 512, (s + 1) * 512)
                    nc.vector.tensor_mul(
                        scaled[:, h * 1024 + s * 512:h * 1024 + (s + 1) * 512],
                        fout_acc[:, cs], invzbs[s][0:65, :])

            with tc.tile_pool(name="psA0", bufs=1, space="PSUM") as psA0:
                fout_accs[0] = psA0.tile([65, 1024], F32, name="fout0")
                with tc.tile_pool(name="psFS", bufs=1, space="PSUM") as psFS:

                    def fs_mms(ck):
                        p = psFS.tile([128, 66], F32, tag="fs", bufs=2,
                                      name=f"p_fs_{ck}")
                        nc.tensor.matmul(p[:],
                                         h1s_aug[:, ck * 128:(ck + 1) * 128],
                                         t_ws2a, start=True, stop=True)
                        nc.vector.tensor_copy(fselfT[:, ck * 65:(ck + 1) * 65],
                                              p[:, 0:65])

                    def h1s_mms(blk):
                        t_xs = xs_tiles[blk]
                        p = psFS.tile([64, 512], F32, tag="fs", bufs=2,
                                      name=f"p_h1s_{blk}")
                        for a in range(2):
                            nc.tensor.matmul(p[:],
                                             t_ws1t[:, a * 64:(a + 1) * 64],
                                             t_xs[:, a * 512:(a + 1) * 512],
                                             start=(a == 0), stop=(a == 1))
                        nc.vector.tensor_copy(
                            h1s_aug[0:64, blk * 512:blk * 512 + 512], p[:])

                    def fx_mms(blk):
                        pf = psFS.tile([128, 512], F32, tag="fs", bufs=2,
                                       name=f"p_fxL_{blk}")
                        nc.tensor.matmul(pf[:], t_bx2r, ones_r,
                                         start=True, stop=False)
                        for a in range(2):
                            nc.tensor.matmul(
                                pf[:], t_wxe[:, a * 128:(a + 1) * 128],
                                t_xl[:, blk * 1024 + a * 512:
                                     blk * 1024 + a * 512 + 512],
                                start=False, stop=(a == 1))
                        nc.vector.tensor_copy(
                            fx2[:, blk * 512:blk * 512 + 512], pf[:])

                    def fy_band1():
                        # fyc block 1 + H band 1 + u2 band 1 + W passes
                        p = psFS.tile([128, 512], F32, tag="fs", bufs=2,
                                      name="p_fyc_1")
                        nc.tensor.matmul(p[:], t_by2r, ones_r,
                                         start=True, stop=False)
                        for a in range(4):
                            nc.tensor.matmul(
                                p[:], t_wye[:, a * 128:(a + 1) * 128],
                                t_yb[:, 2048 + a * 512:2048 + a * 512 + 512],
                                start=False, stop=(a == 3))
                        nc.vector.tensor_scalar_mul(fyc75[:, 512:1024],
                                                    p[:], 0.75)
                        nc.vector.tensor_scalar_mul(fyc25[:, 512:1024],
                                                    p[:], 0.25)
                        nc.vector.tensor_add(fe[:, 16:32, 0, :],
                                             t1v[:, 16:32, :], t2v[:, 15:31, :])
                        nc.vector.tensor_add(fe[:, 15:31, 1, :],
                                             t1v[:, 15:31, :], t2v[:, 16:32, :])
                        nc.vector.tensor_add(fe[:, 31, 1, :],
                                             t1v[:, 31, :], t2v[:, 31, :])
                        nc.vector.tensor_scalar_mul(u2[:, 31 * 32:2048],
                                                    fyH[:, 31 * 32:2048], 0.25)
                        w_pass(0, b1)
                        w_pass(1, b0)
                        w_pass(1, b1)

                    # h1s blocks / fx blocks / fself chunks paced to the
                    # DMA arrival order; fs chunk ck completes well
                    # before pv(ck) (et bufs give ~6 chunks of slack).
                    H1S_AT = {6: 0, 7: 1, 10: 2, 11: 3, 17: 4, 18: 5,
                              21: 6, 22: 7}
                    FX_AT = {14: 2, 15: 3}
                    FS_AT = {7: (0, 1), 8: (2, 3), 9: (4, 5), 10: (6, 7),
                             11: (8, 9), 12: (10, 11), 13: (12, 13),
                             14: (14, 15), 18: (16, 17), 19: (18, 19),
                             20: (20, 21), 21: (22, 23), 22: (24, 25),
                             23: (26, 27), 24: (28, 29), 25: (30, 31)}

                    def fs_hook(ck):
                        if ck == -1:
                            fy_band1()
                            return
                        if ck in FX_AT:
                            fx_mms(FX_AT[ck])
                        if ck in H1S_AT:
                            h1s_mms(H1S_AT[ck])
                        for c in FS_AT.get(ck, ()):
                            fs_mms(c)

                    with tc.tile_pool(name="psB0", bufs=1,
                                      space="PSUM") as psB0:
                        half_loop(0, psB0, fs_hook, defer=8)

                with tc.tile_pool(name="psA1", bufs=1, space="PSUM") as psA1:
                    fout_accs[1] = psA1.tile([65, 1024], F32, name="fout1")

                    def hook1(ck):
                        if ck == 9:
                            # all of half-0's deferred pvs have drained
                            pre_tail(0)

                    with tc.tile_pool(name="psB1", bufs=1,
                                      space="PSUM") as psB1:
                        half_loop(1, psB1, hook1, defer=8)
                        flush_pvs(0)
                    pre_tail(1)


                    # ====== final tail: up-projection + residual ======
                    dma_engines = [nc.sync, nc.scalar]
                    with tc.tile_pool(name="psC", bufs=1, space="PSUM") as psC:
                        for q in range(4):
                            cs = slice(q * 512, (q + 1) * 512)
                            for a in range(2):
                                p = psC.tile([128, 512], F32, tag="up", bufs=4,
                                             name=f"p_up_{q}_{a}")
                                nc.tensor.matmul(
                                    p[:], t_wupt[:, a * 128:(a + 1) * 128],
                                    scaled[:, cs], start=True, stop=True)
                                out_s = sbM.tile([128, 512], F32, tag="tail",
                                                 bufs=8, name=f"out_s_{q}_{a}")
                                xlv = t_xl[:, q * 1024 + a * 512:
                                           q * 1024 + a * 512 + 512]
                                nc.vector.tensor_add(out_s[:], p[:], xlv)
                                dma_engines[(2 * q + a) % 2].dma_start(
                                    out[:, a * 2048 + q * 512:
                                        a * 2048 + (q + 1) * 512], out_s[:])

            sbP1_cm.__exit__(None, None, None)
            if debug:
                nc.sync.dma_start(d_fy2[:], fy2[:])
                nc.sync.dma_start(d_fx2[:], fx2[:])
                nc.sync.dma_start(d_h1s[:], h1s_aug[:])
                nc.sync.dma_start(d_scaled[:], scaled[:])

    nc.compile()
    return nc


def _key_perm():
    """Key order: all even-w pixels (h-major), then all odd-w pixels."""
    k = np.arange(2048)
    even = (k // 32) * 64 + (k % 32) * 2
    odd = even + 1
    return np.concatenate([even, odd])


def _prep_maps(x, y, W_self1, b_self1, W_self2, b_self2, W_x1, b_x1, W_x2,
               b_x2, W_y1, b_y1, W_y2, b_y2, W_up, b_up):
    f64 = np.float64
    f16 = np.float16

    def fold(W2, b1, b2):
        return (W2.astype(f64) @ b1.astype(f64) + b2.astype(f64)).astype(np.float32)

    ws2a = np.zeros((65, 66), np.float32)
    ws2a[64, 0] = 1.0
    ws2a[0:64, 1:65] = W_self2.T
    ws2a[64, 1:65] = fold(W_self2, b_self1, b_self2)

    ws1t = np.ascontiguousarray(
        W_self1.T.reshape(2, 128, 64).transpose(1, 0, 2).reshape(128, 128))
    wxe = (W_x2.astype(f64) @ W_x1.astype(f64)).T      # [256, 64]
    wxe = np.concatenate(
        [np.tile(wxe.reshape(2, 128, 64)[a], (1, 2)) for a in range(2)], 1)
    wye = (W_y2.astype(f64) @ W_y1.astype(f64)).T      # [512, 64]
    wye = np.concatenate(
        [np.tile(wye.reshape(4, 128, 64)[a], (1, 2)) for a in range(4)], 1)
    wupt = np.ascontiguousarray(
        np.concatenate([b_up.reshape(1, 256), W_up.T], axis=0))
    wp = np.zeros((128, _WEND), f16)
    wp[:, _WYE:_WYE + 512] = wye
    wp[0, _WB:_WB + 128] = np.tile(fold(W_x2, b_x1, b_x2), 2)
    wp[0, _WB + 128:_WB + 256] = np.tile(fold(W_y2, b_y1, b_y2), 2)
    wp[0, _WB + 256:_WB + 768] = 1.0
    wp[:, _WXE:_WXE + 256] = wxe
    wp[0:65, _WUP:_WUP + 256] = wupt
    wp[:, _WS1:_WS1 + 128] = ws1t
    wp[0:65, _WS2A:_WS2A + 66] = ws2a

    _ONES = np.ones((1, 4096), f16)
    perm = _key_perm()
    maps = []
    for b in range(B):
        xf = x[b].reshape(CX, N)                                # [256, 4096]
        xs_h = np.ascontiguousarray(
            xf[:, perm].reshape(2, 128, 8, 512).transpose(1, 2, 0, 3)
            .reshape(128, 8192)).astype(f16)
        yf = y[b].reshape(CY, NYC)
        yb_h = np.ascontiguousarray(
            yf.reshape(4, 128, 2, 512).transpose(1, 2, 0, 3).reshape(128, 4096)
        ).astype(f16)
        for half in range(2):
            xh = xf[:, half * NH:(half + 1) * NH]               # [256, 2048]
            xl_h = np.ascontiguousarray(
                xh.reshape(2, 128, 4, 512).transpose(1, 2, 0, 3).reshape(128, 4096)
            ).astype(f16)
            maps.append({
                "xs": xs_h, "xl": xl_h, "yb": yb_h,
                "wpack": wp, "ones": _ONES,
            })
    return maps


def _run(inputs, trace=False, trace_kwargs=None, debug=False):
    key = ("nc", debug)
    if key not in _CACHE:
        _CACHE[key] = _build(debug=debug)
    nc = _CACHE[key]
    maps = _prep_maps(**inputs)
    res = run_bass_kernel_spmd(nc, maps, list(range(8)), trace=trace,
                               **(trace_kwargs or {}))
    outs = np.empty((B, CX, H, W), np.float32)
    for b in range(B):
        for half in range(2):
            o = res.results[2 * b + half]["out"]                # [128, 4096]
            oh = o.reshape(128, 2, NH).transpose(1, 0, 2).reshape(CX, NH)
            outs[b, :, :, :].reshape(CX, N)[:, half * NH:(half + 1) * NH] = oh
    return outs, res


def kernel(**inputs):
    outs, _ = _run(inputs, trace=False)
    return outs
